# revision 32
# baseline (speedup 1.0000x reference)
"""GGNN (gated graph NN) forward on 8 Trainium2 NeuronCores.

Strategy (node-partitioned, SPMD — one Bass program, per-core data):
  - Nodes are permuted and packed into 8 cores x 20 bins x 96 node-column
    slots such that, for every (bin, etype), the number of in-edges is <= 128.
    This makes the aggregation a fixed static structure: one 128-edge tile per
    (etype, bin).
  - h is kept per-core transposed in SBUF ([128 hid, 2048 node-cols]) for all
    dense matmuls; a row-major bf16 hi/lo pair copy ([15360, 256] bf16,
    hi+lo == fp32 h to ~2^-17) lives in DRAM (AllGathered each step) and is
    the source for per-edge dma_gather.
  - Per step, per etype t: gather h[src] rows for all 20 bins in one
    dma_gather; per bin, a one-hot/count matrix S (host-built, bf16, exact)
    scatter-sums the gathered rows into B_t[d, dst] in PSUM via TensorE
    matmuls (hi and lo accumulated, exact in fp32 PSUM); then
    aT += W_t^T @ B_t.  GRU update runs fully on-chip; graph readout is a
    one-hot matmul + AllReduce.
"""

import numpy as np
import ml_dtypes

import concourse.bacc as bacc
import concourse.mybir as mybir
import concourse.tile as tile
from concourse.masks import make_identity
from concourse.bass_utils import run_bass_kernel_spmd

BF16_NP = ml_dtypes.bfloat16

F32 = mybir.dt.float32
F32R = mybir.dt.float32r
BF16 = mybir.dt.bfloat16
I16 = mybir.dt.int16
AF = mybir.ActivationFunctionType
OP = mybir.AluOpType

HID = 128
USE_F32R = True  # fp32r (tf32-ish) for W/GRU matmuls


class Cfg:
    def __init__(self, n_cores, shard, bin_cols, n_etypes, n_steps, n_graphs,
                 n_classes, in_dim):
        assert shard % 128 == 0 and shard % bin_cols == 0
        self.n_cores = n_cores
        self.shard = shard                  # node slots per core
        self.bin = bin_cols                 # node columns per bin
        self.bins = shard // bin_cols       # bins per core
        assert self.bins % 2 == 0
        self.half_bins = self.bins // 2
        self.half_cols = self.half_bins * bin_cols   # node cols per B-half
        assert self.half_cols <= 1024
        self.ntot = n_cores * shard
        self.vpad = ((shard + 511) // 512) * 512     # aT psum width
        self.jt = shard // 128              # 128-wide transpose tiles per core
        self.T = n_etypes
        self.steps = n_steps
        self.G = n_graphs
        self.C = n_classes
        self.in_dim = in_dim
        self.idxc = self.bins * 128 // 16   # idx cols per etype
        self.gbins = 7                      # bins per dma_gather call
        self.scols = self.T * self.bins * self.bin   # S cols total


CFG_FULL = dict(n_cores=8, shard=1920, bin_cols=96, n_etypes=13, n_steps=6,
                n_graphs=64, n_classes=10, in_dim=100)


# ---------------------------------------------------------------- host prep

def _pack_nodes(deg, cfg, rng_order=None):
    """Assign each node to a (global bin, slot) s.t. per-(bin,etype) in-edge
    count <= 128 and per-bin node count <= cfg.bin. Returns slot_of[node]."""
    N = deg.shape[0]
    nbins = cfg.n_cores * cfg.bins
    assert N <= nbins * cfg.bin, "not enough node slots"
    used_e = np.zeros((nbins, cfg.T), np.int64)
    used_s = np.zeros(nbins, np.int64)
    order = np.lexsort((-deg.sum(1), -deg.max(1)))
    bin_of = np.empty(N, np.int64)
    for v in order:
        dv = deg[v]
        ok = (used_s < cfg.bin) & ((used_e + dv) <= 128).all(1)
        if not ok.any():
            raise RuntimeError("bin packing failed; reduce bin_cols")
        cand = np.nonzero(ok)[0]
        load = (used_e[cand] + dv).max(1) * 1.0 + used_s[cand] * 0.01
        b = cand[np.argmin(load)]
        used_e[b] += dv
        bin_of[v] = b
        used_s[b] += 1
    # slots within each bin in node order
    slot_of = np.empty(N, np.int64)
    fill = np.zeros(nbins, np.int64)
    for v in range(N):
        b = bin_of[v]
        core, lb = b // cfg.bins, b % cfg.bins
        slot_of[v] = core * cfg.shard + lb * cfg.bin + fill[b]
        fill[b] += 1
    return slot_of


def _hi_lo_pair(x):
    hi = x.astype(BF16_NP)
    lo = (x - hi.astype(np.float32)).astype(BF16_NP)
    return np.concatenate([hi, lo], axis=-1)


def make_plan(feat, src, dst, etypes, graph_ids, W_e, b_e, W_ih, W_hh, b_ih,
              b_hh, W_cls, b_cls, cfg):
    N = feat.shape[0]
    T, S_, B_, BINS = cfg.T, cfg.shard, cfg.bin, cfg.bins
    deg = np.zeros((N, T), np.int64)
    np.add.at(deg, (dst, etypes), 1)
    slot_of = _pack_nodes(deg, cfg)

    # --- edge plan ---
    dslot = slot_of[dst]
    sslot = slot_of[src]
    gbin = dslot // B_                       # global bin (slot space is bin-aligned)
    core = dslot // S_
    lbin = gbin - core * BINS
    tile_id = etypes.astype(np.int64) * BINS + lbin      # per-core tile index
    order = np.lexsort((dslot, tile_id, core))
    c_o, t_o, ds_o, ss_o = core[order], tile_id[order], dslot[order], sslot[order]
    # row index within each (core, tile) group
    key = c_o * (T * BINS) + t_o
    boundaries = np.nonzero(np.diff(key))[0] + 1
    starts = np.concatenate([[0], boundaries])
    group_of = np.searchsorted(starts, np.arange(len(key)), side="right") - 1
    row = np.arange(len(key)) - starts[group_of]
    assert row.max() < 128, "edge cap exceeded (packing bug)"

    NC = cfg.n_cores
    S_host = np.zeros((NC, 128, cfg.scols), np.float32)
    idx_lin = np.zeros((NC, T * BINS, 128), np.int64)
    np.add.at(S_host, (c_o, row, t_o * B_ + (ds_o % B_)), 1.0)
    idx_lin[c_o, t_o, row] = ss_o

    # idx wrap: per etype block of bins*128 positions -> [16, idxc]
    idx_lin = idx_lin.reshape(NC, T, BINS * 128)
    wrapped = idx_lin.reshape(NC, T, cfg.idxc, 16).transpose(0, 3, 1, 2)
    idx_host = np.tile(wrapped.reshape(NC, 16, T * cfg.idxc), (1, 8, 1))
    idx_host = np.ascontiguousarray(idx_host).astype(np.int16)

    # --- degree matrix (for b_e bias), per core [T, vpad] ---
    D_host = np.zeros((NC, T, cfg.vpad), np.float32)
    np.add.at(D_host, (core, etypes.astype(np.int64), dslot % S_), 1.0)

    # --- graph one-hot, per core [128, jt*G] ---
    node_of_slot = np.full(cfg.ntot, -1, np.int64)
    node_of_slot[slot_of] = np.arange(N)
    G_host = np.zeros((NC, 128, cfg.jt * cfg.G), np.float32)
    for c in range(NC):
        sl = node_of_slot[c * S_:(c + 1) * S_]
        valid = np.nonzero(sl >= 0)[0]
        j, p = valid // 128, valid % 128
        g = graph_ids[sl[valid]]
        G_host[c, p, j * cfg.G + g] = 1.0

    # --- h0 ---
    h0 = np.zeros((cfg.ntot, HID), np.float32)
    h0[slot_of, :cfg.in_dim] = feat
    h0_pair = _hi_lo_pair(h0)                       # [ntot, 256] bf16
    h0T = np.zeros((NC, 128, cfg.vpad), np.float32)
    for c in range(NC):
        h0T[c, :, :S_] = h0[c * S_:(c + 1) * S_].T

    # --- weights ---
    W_host = np.ascontiguousarray(W_e.transpose(1, 0, 2).reshape(128, T * HID))
    WihT = np.ascontiguousarray(W_ih.T)             # [128, 384]
    WhhT = np.ascontiguousarray(W_hh.T)
    bias4 = np.stack([
        b_ih[0:HID] + b_hh[0:HID],                  # r
        b_ih[HID:2 * HID] + b_hh[HID:2 * HID],      # z
        b_ih[2 * HID:],                             # n (input side)
        b_hh[2 * HID:],                             # n (hidden side)
    ], axis=1).astype(np.float32)                   # [128, 4]
    WclsT = np.ascontiguousarray(W_cls.T).astype(np.float32)   # [128, C]
    bclsG = np.tile(b_cls[None, :], (cfg.G, 1)).astype(np.float32)

    in_maps = []
    for c in range(NC):
        in_maps.append({
            "h0_pair": h0_pair,
            "h0T": h0T[c],
            "S": S_host[c].astype(BF16_NP),
            "idx": idx_host[c],
            "D": D_host[c],
            "G": G_host[c],
            "W": W_host.astype(np.float32),
            "Wih": WihT.astype(np.float32),
            "Whh": WhhT.astype(np.float32),
            "be": np.ascontiguousarray(b_e).astype(np.float32),
            "bias4": bias4,
            "Wcls": WclsT,
            "bcls": bclsG,
        })
    return in_maps


# ---------------------------------------------------------------- bass build

def _window_pieces(cfg, b):
    """Split bin b's 96-col window at 512-boundaries of its B-half tile.
    Returns (half, [(b_off, width, s_off), ...]) with b_off relative to the
    half tile."""
    half = b // cfg.half_bins
    start = (b - half * cfg.half_bins) * cfg.bin
    end = start + cfg.bin
    pieces = []
    cur = start
    while cur < end:
        nxt = min(end, (cur // 512 + 1) * 512)
        pieces.append((cur, nxt - cur, cur - start))
        cur = nxt
    return half, pieces


def _wmm_pieces(cfg, half):
    """aT col ranges for the W_t matmul of one B half: split the half's node
    cols at 512-boundaries of the aT tile. Returns [(at_off, width, b_off)]."""
    lo = half * cfg.half_cols
    hi = lo + cfg.half_cols
    out = []
    cur = lo
    while cur < hi:
        nxt = min(hi, (cur // 512 + 1) * 512)
        out.append((cur, nxt - cur, cur - lo))
        cur = nxt
    return out


def build_nc(cfg):
    nc = bacc.Bacc("TRN2", target_bir_lowering=False, debug=False,
                   num_devices=cfg.n_cores, num_swdge_queues=4)
    T, BINS, B_, VP, JT = cfg.T, cfg.bins, cfg.bin, cfg.vpad, cfg.jt
    NCH = 512  # gru chunk

    d_pair0 = nc.dram_tensor("h0_pair", [cfg.ntot, 2 * HID], BF16, kind="ExternalInput")
    d_h0T = nc.dram_tensor("h0T", [128, VP], F32, kind="ExternalInput")
    d_S = nc.dram_tensor("S", [128, cfg.scols], BF16, kind="ExternalInput")
    d_idx = nc.dram_tensor("idx", [128, T * cfg.idxc], I16, kind="ExternalInput")
    d_D = nc.dram_tensor("D", [T, VP], F32, kind="ExternalInput")
    d_G = nc.dram_tensor("G", [128, JT * cfg.G], F32, kind="ExternalInput")
    d_W = nc.dram_tensor("W", [128, T * HID], F32, kind="ExternalInput")
    d_Wih = nc.dram_tensor("Wih", [128, 3 * HID], F32, kind="ExternalInput")
    d_Whh = nc.dram_tensor("Whh", [128, 3 * HID], F32, kind="ExternalInput")
    d_be = nc.dram_tensor("be", [T, HID], F32, kind="ExternalInput")
    d_bias4 = nc.dram_tensor("bias4", [128, 4], F32, kind="ExternalInput")
    d_Wcls = nc.dram_tensor("Wcls", [128, cfg.C], F32, kind="ExternalInput")
    d_bcls = nc.dram_tensor("bcls", [cfg.G, cfg.C], F32, kind="ExternalInput")
    d_out = nc.dram_tensor("out", [cfg.G, cfg.C], F32, kind="ExternalOutput")

    # internal dram (collective bounce, double-buffered)
    aspace = "Shared" if cfg.n_cores > 4 else "Local"
    cc_in = [nc.dram_tensor(f"cc_in{i}", [cfg.shard, 2 * HID], BF16)
             for i in range(2)]
    cc_out = [nc.dram_tensor(f"cc_out{i}", [cfg.ntot, 2 * HID], BF16,
                             addr_space=aspace) for i in range(2)]
    hg_in = nc.dram_tensor("hg_in", [cfg.G, HID], F32)
    hg_out = nc.dram_tensor("hg_out", [cfg.G, HID], F32, addr_space=aspace)

    MMDT = F32R if USE_F32R else F32

    with tile.TileContext(nc) as tc:
        def sb(name, shape, dt=F32):
            return nc.alloc_sbuf_tensor(name, list(shape), dt).ap()

        def ps(name, shape, dt=F32):
            return nc.alloc_psum_tensor(name, list(shape), dt).ap()

        S_sb = sb("S_sb", [128, cfg.scols], BF16)
        idx_sb = sb("idx_sb", [128, T * cfg.idxc], I16)
        hT = sb("hT", [128, VP])
        aT_sb = sb("aT_sb", [128, VP], MMDT)
        W_sb = sb("W_sb", [128, T * HID], MMDT)
        Wih_sb = sb("Wih_sb", [128, 3 * HID], MMDT)
        Whh_sb = sb("Whh_sb", [128, 3 * HID], MMDT)
        be_sb = sb("be_sb", [T, HID], MMDT)
        D_sb = sb("D_sb", [T, VP], MMDT)
        bias_sb = sb("bias_sb", [128, 4])
        G_sb = sb("G_sb", [128, JT * cfg.G])
        Wcls_sb = sb("Wcls_sb", [128, cfg.C])
        bcls_sb = sb("bcls_sb", [cfg.G, cfg.C])
        ident = sb("ident", [128, 128])
        h_rows = sb("h_rows", [128, JT * 128])
        hi32 = sb("hi32", [128, JT * 128])
        pair_sb = sb("pair_sb", [128, JT * 2 * HID], BF16)
        hg_sb = sb("hg_sb", [cfg.G, HID])
        hgT_sb = sb("hgT_sb", [128, cfg.G])
        out_sb = sb("out_sb", [cfg.G, cfg.C])
        hTr = sb("hTr", [128, VP], F32R) if USE_F32R else None

        gbuf = [sb(f"gbuf{i}", [128, BINS * 2 * HID], BF16) for i in range(2)]
        Bsb = [sb(f"Bsb{i}", [128, cfg.half_cols], MMDT) for i in range(2)]
        # GRU scratch, 2 sets alternating by chunk parity
        gsc = [{nm: sb(f"gsc{i}_{nm}", [128, NCH])
                for nm in ("r", "z", "hn", "t1", "t2", "n", "d1", "d2")}
               for i in range(2)]

        B_ps = [ps(f"B_ps{i}", [128, 1024]) for i in range(2)]
        aT_ps = ps("aT_ps", [128, VP])

        # ---------------- setup loads ----------------
        if USE_F32R:
            # load fp32 into staging then round via DVE copy into f32r tiles
            stage = sb("stage", [128, T * HID])
            nc.sync.dma_start(stage[:], d_W[:])
            nc.vector.tensor_copy(W_sb[:], stage[:])
            stage2 = sb("stage2", [128, 3 * HID])
            nc.sync.dma_start(stage2[:], d_Wih[:])
            nc.vector.tensor_copy(Wih_sb[:], stage2[:])
            stage3 = sb("stage3", [128, 3 * HID])
            nc.sync.dma_start(stage3[:], d_Whh[:])
            nc.vector.tensor_copy(Whh_sb[:], stage3[:])
            stage4 = sb("stage4", [T, HID])
            nc.sync.dma_start(stage4[:], d_be[:])
            nc.vector.tensor_copy(be_sb[:], stage4[:])
            stage5 = sb("stage5", [T, VP])
            nc.sync.dma_start(stage5[:], d_D[:])
            nc.vector.tensor_copy(D_sb[:], stage5[:])
        else:
            nc.sync.dma_start(W_sb[:], d_W[:])
            nc.sync.dma_start(Wih_sb[:], d_Wih[:])
            nc.sync.dma_start(Whh_sb[:], d_Whh[:])
            nc.sync.dma_start(be_sb[:], d_be[:])
            nc.sync.dma_start(D_sb[:], d_D[:])
        nc.sync.dma_start(idx_sb[:], d_idx[:])
        SC = BINS * B_
        for t in range(T):
            nc.sync.dma_start(S_sb[:, t * SC:(t + 1) * SC],
                              d_S[:, t * SC:(t + 1) * SC])
        nc.sync.dma_start(hT[:], d_h0T[:])
        nc.sync.dma_start(bias_sb[:], d_bias4[:])
        nc.sync.dma_start(G_sb[:], d_G[:])
        nc.sync.dma_start(Wcls_sb[:], d_Wcls[:])
        nc.sync.dma_start(bcls_sb[:], d_bcls[:])
        make_identity(nc, ident[:])

        # ---------------- steps ----------------
        gq = [0]  # rotating SWDGE queue for gathers
        for s in range(cfg.steps):
            pair_src = d_pair0 if s == 0 else cc_out[s % 2]

            # deg * b_e bias: aT = be^T @ D  (start=True covers all of aT)
            for c0 in range(0, VP, 512):
                nc.tensor.matmul(aT_ps[:, c0:c0 + 512], be_sb[:],
                                 D_sb[:, c0:c0 + 512], start=True, stop=False)

            for t in range(T):
                g = gbuf[t % 2]
                g3 = g[:].rearrange("p (b d) -> p b d", d=2 * HID)
                GB = cfg.gbins
                for b0 in range(0, BINS, GB):
                    nb = min(GB, BINS - b0)
                    nc.gpsimd.dma_gather(
                        g3[:, b0:b0 + nb, :], pair_src[:],
                        idx_sb[:, t * cfg.idxc + b0 * 8:
                               t * cfg.idxc + (b0 + nb) * 8],
                        nb * 128, nb * 128, 2 * HID,
                        queue_num=gq[0] % 4)
                    gq[0] += 1
                for half in range(2):
                    Bp = B_ps[half]
                    # flat entry list: (bank, b_off, w, s_col, lohalf)
                    entries = []
                    for bi in range(cfg.half_bins):
                        b = half * cfg.half_bins + bi
                        _, pieces = _window_pieces(cfg, b)
                        sbase = (t * BINS + b) * B_
                        for (b_off, w, s_off) in pieces:
                            for lo in (0, 1):
                                entries.append((b_off // 512, b_off, w,
                                                sbase + s_off, lo))
                    first_of = {}
                    last_of = {}
                    for i, e in enumerate(entries):
                        first_of.setdefault(e[0], i)
                        last_of[e[0]] = i
                    for i, (bank, b_off, w, s_col, lo) in enumerate(entries):
                        nc.tensor.matmul(
                            Bp[:, b_off:b_off + w],
                            g3[:, (b_off + half * cfg.half_cols) // B_,
                               lo * HID:(lo + 1) * HID],
                            S_sb[:, s_col:s_col + w],
                            start=(first_of[bank] == i),
                            stop=(last_of[bank] == i))
                    # PSUM -> SBUF (rounds to f32r when enabled)
                    if (t * 2 + half) % 2:
                        nc.scalar.activation(Bsb[half][:, :], Bp[:, :cfg.half_cols],
                                             AF.Identity)
                    else:
                        nc.vector.tensor_copy(Bsb[half][:, :], Bp[:, :cfg.half_cols])
                    # aT += W_t^T @ B_half.  stop=True only on the final
                    # accumulation touching each aT bank (t==T-1; for banks
                    # shared by both halves, only half 1's piece closes it).
                    lhsW = W_sb[:, t * HID:(t + 1) * HID]
                    for (at_off, w, b_off) in _wmm_pieces(cfg, half):
                        bank_end = (at_off + w - 1) // 512
                        shared = (cfg.half_cols % 512) != 0 and \
                            bank_end == cfg.half_cols // 512
                        is_stop = (t == T - 1) and not (half == 0 and shared)
                        nc.tensor.matmul(aT_ps[:, at_off:at_off + w], lhsW,
                                         Bsb[half][:, b_off:b_off + w],
                                         start=False, stop=is_stop)

            gru_rhs_h = hTr if USE_F32R else hT

            # aT psum -> sbuf (chunked; must all finish before odd GRU
            # chunks reuse aT_ps banks for gates)
            for c0 in range(0, VP, NCH):
                nc.scalar.activation(aT_sb[:, c0:c0 + NCH],
                                     aT_ps[:, c0:c0 + NCH], AF.Identity)
                if USE_F32R:
                    nc.vector.tensor_copy(hTr[:, c0:c0 + NCH], hT[:, c0:c0 + NCH])

            # ---------------- GRU ----------------
            for ci, c0 in enumerate(range(0, VP, NCH)):
                if ci % 2 == 0:
                    gA, gB = B_ps[0], B_ps[1]
                else:
                    gA, gB = aT_ps[:, 0:2 * NCH], aT_ps[:, 2 * NCH:4 * NCH]
                r_ps, z_ps = gA[:, 0:NCH], gA[:, NCH:2 * NCH]
                ni_ps, nh_ps = gB[:, 0:NCH], gB[:, NCH:2 * NCH]
                a_c = aT_sb[:, c0:c0 + NCH]
                h_c = gru_rhs_h[:, c0:c0 + NCH]
                nc.tensor.matmul(r_ps, Wih_sb[:, 0:HID], a_c, start=True, stop=False)
                nc.tensor.matmul(r_ps, Whh_sb[:, 0:HID], h_c, start=False, stop=True)
                nc.tensor.matmul(z_ps, Wih_sb[:, HID:2 * HID], a_c, start=True, stop=False)
                nc.tensor.matmul(z_ps, Whh_sb[:, HID:2 * HID], h_c, start=False, stop=True)
                nc.tensor.matmul(ni_ps, Wih_sb[:, 2 * HID:3 * HID], a_c, start=True, stop=True)
                nc.tensor.matmul(nh_ps, Whh_sb[:, 2 * HID:3 * HID], h_c, start=True, stop=True)

                sc = gsc[ci % 2]
                r_sb, z_sb, hn_sb = sc["r"], sc["z"], sc["hn"]
                t1_sb, t2_sb, n_sb = sc["t1"], sc["t2"], sc["n"]
                d1_sb, d2_sb = sc["d1"], sc["d2"]
                nc.scalar.activation(r_sb[:], r_ps, AF.Sigmoid, bias=bias_sb[:, 0:1])
                nc.scalar.activation(z_sb[:], z_ps, AF.Sigmoid, bias=bias_sb[:, 1:2])
                nc.scalar.activation(hn_sb[:], nh_ps, AF.Identity, bias=bias_sb[:, 3:4])
                nc.vector.tensor_tensor(out=t1_sb[:], in0=r_sb[:], in1=hn_sb[:], op=OP.mult)
                nc.vector.tensor_tensor(out=t2_sb[:], in0=t1_sb[:], in1=ni_ps, op=OP.add)
                nc.scalar.activation(n_sb[:], t2_sb[:], AF.Tanh, bias=bias_sb[:, 2:3])
                nc.vector.tensor_tensor(out=d1_sb[:], in0=hT[:, c0:c0 + NCH], in1=n_sb[:], op=OP.subtract)
                nc.vector.tensor_tensor(out=d2_sb[:], in0=d1_sb[:], in1=z_sb[:], op=OP.mult)
                nc.vector.tensor_tensor(out=hT[:, c0:c0 + NCH], in0=d2_sb[:], in1=n_sb[:], op=OP.add)

            # ------- transpose h -> rows; hi/lo split + DMA per 512-range ---
            tp_slots = [B_ps[0][:, 0:128], B_ps[1][:, 0:128],
                        aT_ps[:, 0:128]]
            if VP >= 2048:
                tp_slots.append(aT_ps[:, 1024:1152])
            hr3 = h_rows[:].rearrange("p (j d) -> p j d", d=128)
            pr3 = pair_sb[:].rearrange("p (j d) -> p j d", d=2 * HID)
            hi3 = hi32[:].rearrange("p (j d) -> p j d", d=128)
            dst = cc_in[(s + 1) % 2]
            dst3 = dst[:].rearrange("(j p) d -> p j d", p=128)
            j_done = 0
            for j in range(JT):
                tp = tp_slots[j % len(tp_slots)]
                nc.tensor.transpose(tp, hT[:, j * 128:(j + 1) * 128], ident[:])
                if j % 2:
                    nc.scalar.activation(h_rows[:, j * 128:(j + 1) * 128], tp, AF.Identity)
                else:
                    nc.vector.tensor_copy(h_rows[:, j * 128:(j + 1) * 128], tp)
                rng_end = (j + 1) * 128
                if s < cfg.steps - 1 and (rng_end % NCH == 0 or j == JT - 1):
                    j0, j1 = j_done, j + 1
                    j_done = j + 1
                    nc.scalar.activation(pr3[:, j0:j1, 0:HID],
                                         hr3[:, j0:j1, :], AF.Identity)
                    nc.scalar.activation(hi3[:, j0:j1, :], pr3[:, j0:j1, 0:HID],
                                         AF.Identity)
                    nc.vector.tensor_tensor(
                        out=hr3[:, j0:j1, :], in0=hr3[:, j0:j1, :],
                        in1=hi3[:, j0:j1, :], op=OP.subtract)
                    nc.vector.tensor_copy(pr3[:, j0:j1, HID:2 * HID],
                                          hr3[:, j0:j1, :])
                    nc.sync.dma_start(dst3[:, j0:j1, :], pr3[:, j0:j1, :])
            if s < cfg.steps - 1:
                nc.gpsimd.collective_compute(
                    "AllGather", OP.bypass,
                    ins=[dst[:]], outs=[cc_out[(s + 1) % 2][:]],
                    replica_groups=[list(range(cfg.n_cores))])

        # ---------------- readout ----------------
        hg_ps = B_ps[0][0:cfg.G, 0:HID]
        for j in range(JT):
            nc.tensor.matmul(hg_ps, G_sb[:, j * cfg.G:(j + 1) * cfg.G],
                             h_rows[:, j * 128:(j + 1) * 128],
                             start=(j == 0), stop=(j == JT - 1))
        nc.scalar.activation(hg_sb[:], hg_ps, AF.Identity)
        nc.sync.dma_start(hg_in[:], hg_sb[:])
        nc.gpsimd.collective_compute(
            "AllReduce", OP.add, ins=[hg_in[:]], outs=[hg_out[:]],
            replica_groups=[list(range(cfg.n_cores))])
        hg_all = sb("hg_all", [cfg.G, HID])
        nc.sync.dma_start(hg_all[:], hg_out[:])
        tp_ps = B_ps[1][:, 0:cfg.G]
        nc.tensor.transpose(tp_ps, hg_all[:], ident[0:cfg.G, 0:cfg.G])
        nc.vector.tensor_copy(hgT_sb[:], tp_ps)
        lg_ps = B_ps[0][0:cfg.G, 512:512 + cfg.C]
        nc.tensor.matmul(lg_ps, hgT_sb[:], Wcls_sb[:], start=True, stop=True)
        nc.vector.tensor_tensor(out=out_sb[:], in0=lg_ps, in1=bcls_sb[:], op=OP.add)
        nc.sync.dma_start(d_out[:], out_sb[:])

    nc.compile()
    return nc


# ---------------------------------------------------------------- entry

_CACHE = {}
LAST_EXEC_NS = None
LAST_RESULTS = None
PROFILE = False


def _get_nc(cfg_key, cfg):
    if cfg_key not in _CACHE:
        _CACHE[cfg_key] = build_nc(cfg)
    return _CACHE[cfg_key]


def kernel(feat, src, dst, etypes, graph_ids, W_e, b_e, W_ih, W_hh, b_ih,
           b_hh, W_cls, b_cls):
    feat = np.asarray(feat, np.float32)
    args = dict(src=np.asarray(src), dst=np.asarray(dst),
                etypes=np.asarray(etypes), graph_ids=np.asarray(graph_ids),
                W_e=np.asarray(W_e, np.float32), b_e=np.asarray(b_e, np.float32),
                W_ih=np.asarray(W_ih, np.float32), W_hh=np.asarray(W_hh, np.float32),
                b_ih=np.asarray(b_ih, np.float32), b_hh=np.asarray(b_hh, np.float32),
                W_cls=np.asarray(W_cls, np.float32), b_cls=np.asarray(b_cls, np.float32))
    cfg = Cfg(**CFG_FULL)
    in_maps = make_plan(feat=feat, cfg=cfg, **args)
    nc = _get_nc("full", cfg)
    res = run_bass_kernel_spmd(nc, in_maps, list(range(cfg.n_cores)),
                               trace=PROFILE)
    global LAST_EXEC_NS, LAST_RESULTS
    LAST_EXEC_NS = res.exec_time_ns
    LAST_RESULTS = res
    return np.asarray(res.results[0]["out"], np.float32)


# revision 33
# speedup vs baseline: 1.0180x; 1.0180x over previous
"""GGNN (gated graph NN) forward on 8 Trainium2 NeuronCores.

Strategy (node-partitioned, SPMD — one Bass program, per-core data):
  - Nodes are permuted and packed into 8 cores x 20 bins x 96 node-column
    slots such that, for every (bin, etype), the number of in-edges is <= 128.
    This makes the aggregation a fixed static structure: one 128-edge tile per
    (etype, bin).
  - h is kept per-core transposed in SBUF ([128 hid, 2048 node-cols]) for all
    dense matmuls; a row-major bf16 hi/lo pair copy ([15360, 256] bf16,
    hi+lo == fp32 h to ~2^-17) lives in DRAM (AllGathered each step) and is
    the source for per-edge dma_gather.
  - Per step, per etype t: gather h[src] rows (dma_gather, 7 bins = 896 rows
    per call, round-robin over 4 SWDGE queues; >1024-row calls crash the Q7
    ring); per bin, a one-hot/count matrix S (host-built, bf16, exact)
    scatter-sums the gathered rows into B_t[d, dst] in PSUM via TensorE
    matmuls (hi and lo accumulated, exact in fp32 PSUM); then
    aT += W_t^T @ B_t (float32r).  GRU update runs fully on-chip (gates in
    PSUM, sigmoid/tanh on ACT with per-partition bias); graph readout is a
    one-hot matmul + AllReduce.
  - Measured on 8 trn2 cores: ~1.33 ms, rel err 5.7e-4 vs fp32 reference
    (float32r W/GRU matmuls dominate the error; USE_F32R=False gives 6e-6
    at ~2.4 ms).
"""

import numpy as np
import ml_dtypes

import concourse.bacc as bacc
import concourse.mybir as mybir
import concourse.tile as tile
from concourse.masks import make_identity
from concourse.bass_utils import run_bass_kernel_spmd

BF16_NP = ml_dtypes.bfloat16

F32 = mybir.dt.float32
F32R = mybir.dt.float32r
BF16 = mybir.dt.bfloat16
I16 = mybir.dt.int16
AF = mybir.ActivationFunctionType
OP = mybir.AluOpType

HID = 128
USE_F32R = True  # fp32r (tf32-ish) for W/GRU matmuls


class Cfg:
    def __init__(self, n_cores, shard, bin_cols, n_etypes, n_steps, n_graphs,
                 n_classes, in_dim):
        assert shard % 128 == 0 and shard % bin_cols == 0
        self.n_cores = n_cores
        self.shard = shard                  # node slots per core
        self.bin = bin_cols                 # node columns per bin
        self.bins = shard // bin_cols       # bins per core
        assert self.bins % 2 == 0
        self.half_bins = self.bins // 2
        self.half_cols = self.half_bins * bin_cols   # node cols per B-half
        assert self.half_cols <= 1024
        self.ntot = n_cores * shard
        self.vpad = ((shard + 511) // 512) * 512     # aT psum width
        self.jt = shard // 128              # 128-wide transpose tiles per core
        self.T = n_etypes
        self.steps = n_steps
        self.G = n_graphs
        self.C = n_classes
        self.in_dim = in_dim
        self.idxc = self.bins * 128 // 16   # idx cols per etype
        self.gbins = 7                      # bins per dma_gather call
        self.scols = self.T * self.bins * self.bin   # S cols total


CFG_FULL = dict(n_cores=8, shard=1920, bin_cols=96, n_etypes=13, n_steps=6,
                n_graphs=64, n_classes=10, in_dim=100)


# ---------------------------------------------------------------- host prep

def _pack_nodes(deg, cfg, rng_order=None):
    """Assign each node to a (global bin, slot) s.t. per-(bin,etype) in-edge
    count <= 128 and per-bin node count <= cfg.bin. Returns slot_of[node]."""
    N = deg.shape[0]
    nbins = cfg.n_cores * cfg.bins
    assert N <= nbins * cfg.bin, "not enough node slots"
    used_e = np.zeros((nbins, cfg.T), np.int64)
    used_s = np.zeros(nbins, np.int64)
    order = np.lexsort((-deg.sum(1), -deg.max(1)))
    bin_of = np.empty(N, np.int64)
    for v in order:
        dv = deg[v]
        ok = (used_s < cfg.bin) & ((used_e + dv) <= 128).all(1)
        if not ok.any():
            raise RuntimeError("bin packing failed; reduce bin_cols")
        cand = np.nonzero(ok)[0]
        load = (used_e[cand] + dv).max(1) * 1.0 + used_s[cand] * 0.01
        b = cand[np.argmin(load)]
        used_e[b] += dv
        bin_of[v] = b
        used_s[b] += 1
    # slots within each bin in node order
    slot_of = np.empty(N, np.int64)
    fill = np.zeros(nbins, np.int64)
    for v in range(N):
        b = bin_of[v]
        core, lb = b // cfg.bins, b % cfg.bins
        slot_of[v] = core * cfg.shard + lb * cfg.bin + fill[b]
        fill[b] += 1
    return slot_of


def _hi_lo_pair(x):
    hi = x.astype(BF16_NP)
    lo = (x - hi.astype(np.float32)).astype(BF16_NP)
    return np.concatenate([hi, lo], axis=-1)


def make_plan(feat, src, dst, etypes, graph_ids, W_e, b_e, W_ih, W_hh, b_ih,
              b_hh, W_cls, b_cls, cfg):
    N = feat.shape[0]
    T, S_, B_, BINS = cfg.T, cfg.shard, cfg.bin, cfg.bins
    deg = np.zeros((N, T), np.int64)
    np.add.at(deg, (dst, etypes), 1)
    slot_of = _pack_nodes(deg, cfg)

    # --- edge plan ---
    dslot = slot_of[dst]
    sslot = slot_of[src]
    gbin = dslot // B_                       # global bin (slot space is bin-aligned)
    core = dslot // S_
    lbin = gbin - core * BINS
    tile_id = etypes.astype(np.int64) * BINS + lbin      # per-core tile index
    order = np.lexsort((dslot, tile_id, core))
    c_o, t_o, ds_o, ss_o = core[order], tile_id[order], dslot[order], sslot[order]
    # row index within each (core, tile) group
    key = c_o * (T * BINS) + t_o
    boundaries = np.nonzero(np.diff(key))[0] + 1
    starts = np.concatenate([[0], boundaries])
    group_of = np.searchsorted(starts, np.arange(len(key)), side="right") - 1
    row = np.arange(len(key)) - starts[group_of]
    assert row.max() < 128, "edge cap exceeded (packing bug)"

    NC = cfg.n_cores
    S_host = np.zeros((NC, 128, cfg.scols), np.float32)
    idx_lin = np.zeros((NC, T * BINS, 128), np.int64)
    np.add.at(S_host, (c_o, row, t_o * B_ + (ds_o % B_)), 1.0)
    idx_lin[c_o, t_o, row] = ss_o

    # idx wrap: per etype block of bins*128 positions -> [16, idxc]
    idx_lin = idx_lin.reshape(NC, T, BINS * 128)
    wrapped = idx_lin.reshape(NC, T, cfg.idxc, 16).transpose(0, 3, 1, 2)
    idx_host = np.tile(wrapped.reshape(NC, 16, T * cfg.idxc), (1, 8, 1))
    idx_host = np.ascontiguousarray(idx_host).astype(np.int16)

    # --- degree matrix (for b_e bias), per core [T, vpad] ---
    D_host = np.zeros((NC, T, cfg.vpad), np.float32)
    np.add.at(D_host, (core, etypes.astype(np.int64), dslot % S_), 1.0)

    # --- graph one-hot, per core [128, jt*G] ---
    node_of_slot = np.full(cfg.ntot, -1, np.int64)
    node_of_slot[slot_of] = np.arange(N)
    G_host = np.zeros((NC, 128, cfg.jt * cfg.G), np.float32)
    for c in range(NC):
        sl = node_of_slot[c * S_:(c + 1) * S_]
        valid = np.nonzero(sl >= 0)[0]
        j, p = valid // 128, valid % 128
        g = graph_ids[sl[valid]]
        G_host[c, p, j * cfg.G + g] = 1.0

    # --- h0 ---
    h0 = np.zeros((cfg.ntot, HID), np.float32)
    h0[slot_of, :cfg.in_dim] = feat
    h0_pair = _hi_lo_pair(h0)                       # [ntot, 256] bf16
    h0T = np.zeros((NC, 128, cfg.vpad), np.float32)
    for c in range(NC):
        h0T[c, :, :S_] = h0[c * S_:(c + 1) * S_].T

    # --- weights ---
    W_host = np.ascontiguousarray(W_e.transpose(1, 0, 2).reshape(128, T * HID))
    WihT = np.ascontiguousarray(W_ih.T)             # [128, 384]
    WhhT = np.ascontiguousarray(W_hh.T)
    bias4 = np.stack([
        b_ih[0:HID] + b_hh[0:HID],                  # r
        b_ih[HID:2 * HID] + b_hh[HID:2 * HID],      # z
        b_ih[2 * HID:],                             # n (input side)
        b_hh[2 * HID:],                             # n (hidden side)
    ], axis=1).astype(np.float32)                   # [128, 4]
    WclsT = np.ascontiguousarray(W_cls.T).astype(np.float32)   # [128, C]
    bclsG = np.tile(b_cls[None, :], (cfg.G, 1)).astype(np.float32)

    in_maps = []
    for c in range(NC):
        in_maps.append({
            "h0_pair": h0_pair,
            "h0T": h0T[c],
            "S": S_host[c].astype(BF16_NP),
            "idx": idx_host[c],
            "D": D_host[c],
            "G": G_host[c],
            "W": W_host.astype(np.float32),
            "Wih": WihT.astype(np.float32),
            "Whh": WhhT.astype(np.float32),
            "be": np.ascontiguousarray(b_e).astype(np.float32),
            "bias4": bias4,
            "Wcls": WclsT,
            "bcls": bclsG,
        })
    return in_maps


# ---------------------------------------------------------------- bass build

def _window_pieces(cfg, b):
    """Split bin b's 96-col window at 512-boundaries of its B-half tile.
    Returns (half, [(b_off, width, s_off), ...]) with b_off relative to the
    half tile."""
    half = b // cfg.half_bins
    start = (b - half * cfg.half_bins) * cfg.bin
    end = start + cfg.bin
    pieces = []
    cur = start
    while cur < end:
        nxt = min(end, (cur // 512 + 1) * 512)
        pieces.append((cur, nxt - cur, cur - start))
        cur = nxt
    return half, pieces


def _wmm_pieces(cfg, half):
    """aT col ranges for the W_t matmul of one B half: split the half's node
    cols at 512-boundaries of the aT tile. Returns [(at_off, width, b_off)]."""
    lo = half * cfg.half_cols
    hi = lo + cfg.half_cols
    out = []
    cur = lo
    while cur < hi:
        nxt = min(hi, (cur // 512 + 1) * 512)
        out.append((cur, nxt - cur, cur - lo))
        cur = nxt
    return out


def build_nc(cfg):
    nc = bacc.Bacc("TRN2", target_bir_lowering=False, debug=False,
                   num_devices=cfg.n_cores, num_swdge_queues=4)
    T, BINS, B_, VP, JT = cfg.T, cfg.bins, cfg.bin, cfg.vpad, cfg.jt
    NCH = 512  # gru chunk

    d_pair0 = nc.dram_tensor("h0_pair", [cfg.ntot, 2 * HID], BF16, kind="ExternalInput")
    d_h0T = nc.dram_tensor("h0T", [128, VP], F32, kind="ExternalInput")
    d_S = nc.dram_tensor("S", [128, cfg.scols], BF16, kind="ExternalInput")
    d_idx = nc.dram_tensor("idx", [128, T * cfg.idxc], I16, kind="ExternalInput")
    d_D = nc.dram_tensor("D", [T, VP], F32, kind="ExternalInput")
    d_G = nc.dram_tensor("G", [128, JT * cfg.G], F32, kind="ExternalInput")
    d_W = nc.dram_tensor("W", [128, T * HID], F32, kind="ExternalInput")
    d_Wih = nc.dram_tensor("Wih", [128, 3 * HID], F32, kind="ExternalInput")
    d_Whh = nc.dram_tensor("Whh", [128, 3 * HID], F32, kind="ExternalInput")
    d_be = nc.dram_tensor("be", [T, HID], F32, kind="ExternalInput")
    d_bias4 = nc.dram_tensor("bias4", [128, 4], F32, kind="ExternalInput")
    d_Wcls = nc.dram_tensor("Wcls", [128, cfg.C], F32, kind="ExternalInput")
    d_bcls = nc.dram_tensor("bcls", [cfg.G, cfg.C], F32, kind="ExternalInput")
    d_out = nc.dram_tensor("out", [cfg.G, cfg.C], F32, kind="ExternalOutput")

    # internal dram (collective bounce, double-buffered)
    aspace = "Shared" if cfg.n_cores > 4 else "Local"
    cc_in = [nc.dram_tensor(f"cc_in{i}", [cfg.shard, 2 * HID], BF16)
             for i in range(2)]
    cc_out = [nc.dram_tensor(f"cc_out{i}", [cfg.ntot, 2 * HID], BF16,
                             addr_space=aspace) for i in range(2)]
    hg_in = nc.dram_tensor("hg_in", [cfg.G, HID], F32)
    hg_out = nc.dram_tensor("hg_out", [cfg.G, HID], F32, addr_space=aspace)

    MMDT = F32R if USE_F32R else F32

    with tile.TileContext(nc) as tc:
        def sb(name, shape, dt=F32):
            return nc.alloc_sbuf_tensor(name, list(shape), dt).ap()

        def ps(name, shape, dt=F32):
            return nc.alloc_psum_tensor(name, list(shape), dt).ap()

        S_sb = sb("S_sb", [128, cfg.scols], BF16)
        idx_sb = sb("idx_sb", [128, T * cfg.idxc], I16)
        hT = sb("hT", [128, VP])
        aT_sb = sb("aT_sb", [128, VP], MMDT)
        W_sb = sb("W_sb", [128, T * HID], MMDT)
        Wih_sb = sb("Wih_sb", [128, 3 * HID], MMDT)
        Whh_sb = sb("Whh_sb", [128, 3 * HID], MMDT)
        be_sb = sb("be_sb", [T, HID], MMDT)
        D_sb = sb("D_sb", [T, VP], MMDT)
        bias_sb = sb("bias_sb", [128, 4])
        G_sb = sb("G_sb", [128, JT * cfg.G])
        Wcls_sb = sb("Wcls_sb", [128, cfg.C])
        bcls_sb = sb("bcls_sb", [cfg.G, cfg.C])
        ident = sb("ident", [128, 128])
        h_rows = sb("h_rows", [128, JT * 128])
        hi32 = sb("hi32", [128, JT * 128])
        pair_sb = sb("pair_sb", [128, JT * 2 * HID], BF16)
        hg_sb = sb("hg_sb", [cfg.G, HID])
        hgT_sb = sb("hgT_sb", [128, cfg.G])
        out_sb = sb("out_sb", [cfg.G, cfg.C])
        hTr = sb("hTr", [128, VP], F32R) if USE_F32R else None

        gbuf = [sb(f"gbuf{i}", [128, BINS * 2 * HID], BF16) for i in range(2)]
        Bsb = [sb(f"Bsb{i}", [128, cfg.half_cols], MMDT) for i in range(2)]
        # GRU scratch, 2 sets alternating by chunk parity
        gsc = [{nm: sb(f"gsc{i}_{nm}", [128, NCH])
                for nm in ("r", "z", "hn", "t1", "t2", "n", "d1", "d2")}
               for i in range(2)]

        B_ps = [ps(f"B_ps{i}", [128, 1024]) for i in range(2)]
        aT_ps = ps("aT_ps", [128, VP])

        # ---------------- setup loads ----------------
        if USE_F32R:
            # load fp32 into staging then round via DVE copy into f32r tiles
            stage = sb("stage", [128, T * HID])
            nc.sync.dma_start(stage[:], d_W[:])
            nc.vector.tensor_copy(W_sb[:], stage[:])
            stage2 = sb("stage2", [128, 3 * HID])
            nc.sync.dma_start(stage2[:], d_Wih[:])
            nc.vector.tensor_copy(Wih_sb[:], stage2[:])
            stage3 = sb("stage3", [128, 3 * HID])
            nc.sync.dma_start(stage3[:], d_Whh[:])
            nc.vector.tensor_copy(Whh_sb[:], stage3[:])
            stage4 = sb("stage4", [T, HID])
            nc.sync.dma_start(stage4[:], d_be[:])
            nc.vector.tensor_copy(be_sb[:], stage4[:])
            stage5 = sb("stage5", [T, VP])
            nc.sync.dma_start(stage5[:], d_D[:])
            nc.vector.tensor_copy(D_sb[:], stage5[:])
        else:
            nc.sync.dma_start(W_sb[:], d_W[:])
            nc.sync.dma_start(Wih_sb[:], d_Wih[:])
            nc.sync.dma_start(Whh_sb[:], d_Whh[:])
            nc.sync.dma_start(be_sb[:], d_be[:])
            nc.sync.dma_start(D_sb[:], d_D[:])
        nc.sync.dma_start(idx_sb[:], d_idx[:])
        SC = BINS * B_
        for t in range(T):
            nc.sync.dma_start(S_sb[:, t * SC:(t + 1) * SC],
                              d_S[:, t * SC:(t + 1) * SC])
        nc.sync.dma_start(hT[:], d_h0T[:])
        nc.sync.dma_start(bias_sb[:], d_bias4[:])
        nc.sync.dma_start(G_sb[:], d_G[:])
        nc.sync.dma_start(Wcls_sb[:], d_Wcls[:])
        nc.sync.dma_start(bcls_sb[:], d_bcls[:])
        make_identity(nc, ident[:])

        # ---------------- steps ----------------
        gq = [0]  # rotating SWDGE queue for gathers
        for s in range(cfg.steps):
            pair_src = d_pair0 if s == 0 else cc_out[s % 2]

            # deg * b_e bias: aT = be^T @ D  (start=True covers all of aT)
            for c0 in range(0, VP, 512):
                nc.tensor.matmul(aT_ps[:, c0:c0 + 512], be_sb[:],
                                 D_sb[:, c0:c0 + 512], start=True, stop=False)

            for t in range(T):
                g = gbuf[t % 2]
                g3 = g[:].rearrange("p (b d) -> p b d", d=2 * HID)
                GB = cfg.gbins
                for b0 in range(0, BINS, GB):
                    nb = min(GB, BINS - b0)
                    nc.gpsimd.dma_gather(
                        g3[:, b0:b0 + nb, :], pair_src[:],
                        idx_sb[:, t * cfg.idxc + b0 * 8:
                               t * cfg.idxc + (b0 + nb) * 8],
                        nb * 128, nb * 128, 2 * HID,
                        queue_num=gq[0] % 4)
                    gq[0] += 1
                for half in range(2):
                    Bp = B_ps[half]
                    # flat entry list: (bank, b_off, w, s_col, lohalf)
                    entries = []
                    for bi in range(cfg.half_bins):
                        b = half * cfg.half_bins + bi
                        _, pieces = _window_pieces(cfg, b)
                        sbase = (t * BINS + b) * B_
                        for (b_off, w, s_off) in pieces:
                            for lo in (0, 1):
                                entries.append((b_off // 512, b_off, w,
                                                sbase + s_off, lo))
                    first_of = {}
                    last_of = {}
                    for i, e in enumerate(entries):
                        first_of.setdefault(e[0], i)
                        last_of[e[0]] = i
                    for i, (bank, b_off, w, s_col, lo) in enumerate(entries):
                        nc.tensor.matmul(
                            Bp[:, b_off:b_off + w],
                            g3[:, (b_off + half * cfg.half_cols) // B_,
                               lo * HID:(lo + 1) * HID],
                            S_sb[:, s_col:s_col + w],
                            start=(first_of[bank] == i),
                            stop=(last_of[bank] == i))
                    # PSUM -> SBUF (rounds to f32r when enabled)
                    if (t * 2 + half) % 2:
                        nc.scalar.activation(Bsb[half][:, :], Bp[:, :cfg.half_cols],
                                             AF.Identity)
                    else:
                        nc.vector.tensor_copy(Bsb[half][:, :], Bp[:, :cfg.half_cols])
                    # aT += W_t^T @ B_half.  stop=True only on the final
                    # accumulation touching each aT bank (t==T-1; for banks
                    # shared by both halves, only half 1's piece closes it).
                    lhsW = W_sb[:, t * HID:(t + 1) * HID]
                    for (at_off, w, b_off) in _wmm_pieces(cfg, half):
                        bank_end = (at_off + w - 1) // 512
                        shared = (cfg.half_cols % 512) != 0 and \
                            bank_end == cfg.half_cols // 512
                        is_stop = (t == T - 1) and not (half == 0 and shared)
                        nc.tensor.matmul(aT_ps[:, at_off:at_off + w], lhsW,
                                         Bsb[half][:, b_off:b_off + w],
                                         start=False, stop=is_stop)

            gru_rhs_h = hTr if USE_F32R else hT

            # aT psum -> sbuf (chunked; must all finish before odd GRU
            # chunks reuse aT_ps banks for gates)
            for c0 in range(0, VP, NCH):
                nc.scalar.activation(aT_sb[:, c0:c0 + NCH],
                                     aT_ps[:, c0:c0 + NCH], AF.Identity)
                if USE_F32R:
                    nc.vector.tensor_copy(hTr[:, c0:c0 + NCH], hT[:, c0:c0 + NCH])

            # ---------------- GRU ----------------
            for ci, c0 in enumerate(range(0, VP, NCH)):
                if ci % 2 == 0:
                    gA, gB = B_ps[0], B_ps[1]
                else:
                    gA, gB = aT_ps[:, 0:2 * NCH], aT_ps[:, 2 * NCH:4 * NCH]
                r_ps, z_ps = gA[:, 0:NCH], gA[:, NCH:2 * NCH]
                ni_ps, nh_ps = gB[:, 0:NCH], gB[:, NCH:2 * NCH]
                a_c = aT_sb[:, c0:c0 + NCH]
                h_c = gru_rhs_h[:, c0:c0 + NCH]
                nc.tensor.matmul(r_ps, Wih_sb[:, 0:HID], a_c, start=True, stop=False)
                nc.tensor.matmul(r_ps, Whh_sb[:, 0:HID], h_c, start=False, stop=True)
                nc.tensor.matmul(z_ps, Wih_sb[:, HID:2 * HID], a_c, start=True, stop=False)
                nc.tensor.matmul(z_ps, Whh_sb[:, HID:2 * HID], h_c, start=False, stop=True)
                nc.tensor.matmul(ni_ps, Wih_sb[:, 2 * HID:3 * HID], a_c, start=True, stop=True)
                nc.tensor.matmul(nh_ps, Whh_sb[:, 2 * HID:3 * HID], h_c, start=True, stop=True)

                sc = gsc[ci % 2]
                r_sb, z_sb, hn_sb = sc["r"], sc["z"], sc["hn"]
                t1_sb, t2_sb, n_sb = sc["t1"], sc["t2"], sc["n"]
                d1_sb, d2_sb = sc["d1"], sc["d2"]
                nc.scalar.activation(r_sb[:], r_ps, AF.Sigmoid, bias=bias_sb[:, 0:1])
                nc.scalar.activation(z_sb[:], z_ps, AF.Sigmoid, bias=bias_sb[:, 1:2])
                nc.scalar.activation(hn_sb[:], nh_ps, AF.Identity, bias=bias_sb[:, 3:4])
                nc.vector.tensor_tensor(out=t1_sb[:], in0=r_sb[:], in1=hn_sb[:], op=OP.mult)
                nc.vector.tensor_tensor(out=t2_sb[:], in0=t1_sb[:], in1=ni_ps, op=OP.add)
                nc.scalar.activation(n_sb[:], t2_sb[:], AF.Tanh, bias=bias_sb[:, 2:3])
                nc.vector.tensor_tensor(out=d1_sb[:], in0=hT[:, c0:c0 + NCH], in1=n_sb[:], op=OP.subtract)
                nc.vector.tensor_tensor(out=d2_sb[:], in0=d1_sb[:], in1=z_sb[:], op=OP.mult)
                nc.vector.tensor_tensor(out=hT[:, c0:c0 + NCH], in0=d2_sb[:], in1=n_sb[:], op=OP.add)

            # ------- transpose h -> rows; hi/lo split + DMA per 512-range ---
            tp_slots = [B_ps[0][:, 0:128], B_ps[1][:, 0:128],
                        aT_ps[:, 0:128]]
            if VP >= 2048:
                tp_slots.append(aT_ps[:, 1024:1152])
            hr3 = h_rows[:].rearrange("p (j d) -> p j d", d=128)
            pr3 = pair_sb[:].rearrange("p (j d) -> p j d", d=2 * HID)
            hi3 = hi32[:].rearrange("p (j d) -> p j d", d=128)
            dst = cc_in[(s + 1) % 2]
            dst3 = dst[:].rearrange("(j p) d -> p j d", p=128)
            j_done = 0
            for j in range(JT):
                tp = tp_slots[j % len(tp_slots)]
                nc.tensor.transpose(tp, hT[:, j * 128:(j + 1) * 128], ident[:])
                if j % 2:
                    nc.scalar.activation(h_rows[:, j * 128:(j + 1) * 128], tp, AF.Identity)
                else:
                    nc.vector.tensor_copy(h_rows[:, j * 128:(j + 1) * 128], tp)
                rng_end = (j + 1) * 128
                if s < cfg.steps - 1 and (rng_end % NCH == 0 or j == JT - 1):
                    j0, j1 = j_done, j + 1
                    j_done = j + 1
                    nc.scalar.activation(pr3[:, j0:j1, 0:HID],
                                         hr3[:, j0:j1, :], AF.Identity)
                    nc.scalar.activation(hi3[:, j0:j1, :], pr3[:, j0:j1, 0:HID],
                                         AF.Identity)
                    nc.vector.tensor_tensor(
                        out=hr3[:, j0:j1, :], in0=hr3[:, j0:j1, :],
                        in1=hi3[:, j0:j1, :], op=OP.subtract)
                    nc.vector.tensor_copy(pr3[:, j0:j1, HID:2 * HID],
                                          hr3[:, j0:j1, :])
                    nc.sync.dma_start(dst3[:, j0:j1, :], pr3[:, j0:j1, :])
            if s < cfg.steps - 1:
                nc.gpsimd.collective_compute(
                    "AllGather", OP.bypass,
                    ins=[dst[:]], outs=[cc_out[(s + 1) % 2][:]],
                    replica_groups=[list(range(cfg.n_cores))])

        # ---------------- readout ----------------
        hg_ps = B_ps[0][0:cfg.G, 0:HID]
        for j in range(JT):
            nc.tensor.matmul(hg_ps, G_sb[:, j * cfg.G:(j + 1) * cfg.G],
                             h_rows[:, j * 128:(j + 1) * 128],
                             start=(j == 0), stop=(j == JT - 1))
        nc.scalar.activation(hg_sb[:], hg_ps, AF.Identity)
        nc.sync.dma_start(hg_in[:], hg_sb[:])
        nc.gpsimd.collective_compute(
            "AllReduce", OP.add, ins=[hg_in[:]], outs=[hg_out[:]],
            replica_groups=[list(range(cfg.n_cores))])
        hg_all = sb("hg_all", [cfg.G, HID])
        nc.sync.dma_start(hg_all[:], hg_out[:])
        tp_ps = B_ps[1][:, 0:cfg.G]
        nc.tensor.transpose(tp_ps, hg_all[:], ident[0:cfg.G, 0:cfg.G])
        nc.vector.tensor_copy(hgT_sb[:], tp_ps)
        lg_ps = B_ps[0][0:cfg.G, 512:512 + cfg.C]
        nc.tensor.matmul(lg_ps, hgT_sb[:], Wcls_sb[:], start=True, stop=True)
        nc.vector.tensor_tensor(out=out_sb[:], in0=lg_ps, in1=bcls_sb[:], op=OP.add)
        nc.sync.dma_start(d_out[:], out_sb[:])

    nc.compile()
    return nc


# ---------------------------------------------------------------- entry

_CACHE = {}
LAST_EXEC_NS = None
LAST_RESULTS = None
PROFILE = False


def _get_nc(cfg_key, cfg):
    if cfg_key not in _CACHE:
        _CACHE[cfg_key] = build_nc(cfg)
    return _CACHE[cfg_key]


def kernel(feat, src, dst, etypes, graph_ids, W_e, b_e, W_ih, W_hh, b_ih,
           b_hh, W_cls, b_cls):
    feat = np.asarray(feat, np.float32)
    args = dict(src=np.asarray(src), dst=np.asarray(dst),
                etypes=np.asarray(etypes), graph_ids=np.asarray(graph_ids),
                W_e=np.asarray(W_e, np.float32), b_e=np.asarray(b_e, np.float32),
                W_ih=np.asarray(W_ih, np.float32), W_hh=np.asarray(W_hh, np.float32),
                b_ih=np.asarray(b_ih, np.float32), b_hh=np.asarray(b_hh, np.float32),
                W_cls=np.asarray(W_cls, np.float32), b_cls=np.asarray(b_cls, np.float32))
    cfg = Cfg(**CFG_FULL)
    in_maps = make_plan(feat=feat, cfg=cfg, **args)
    nc = _get_nc("full", cfg)
    res = run_bass_kernel_spmd(nc, in_maps, list(range(cfg.n_cores)),
                               trace=PROFILE)
    global LAST_EXEC_NS, LAST_RESULTS
    LAST_EXEC_NS = res.exec_time_ns
    LAST_RESULTS = res
    return np.asarray(res.results[0]["out"], np.float32)


# revision 35
# speedup vs baseline: 1.2024x; 1.1811x over previous
"""GGNN (gated graph NN) forward on 8 Trainium2 NeuronCores.

Strategy (node-partitioned, SPMD — one Bass program, per-core data):
  - Nodes are permuted and packed into 8 cores x 20 bins x 96 node-column
    slots such that, for every (bin, etype), the number of in-edges is <= 128.
    This makes the aggregation a fixed static structure: one 128-edge tile per
    (etype, bin).
  - h is kept per-core transposed in SBUF ([128 hid, 2048 node-cols]) for all
    dense matmuls; a row-major bf16 hi/lo pair copy ([15360, 256] bf16,
    hi+lo == fp32 h to ~2^-17) lives in DRAM (AllGathered each step) and is
    the source for per-edge dma_gather.
  - Per step, per etype t: gather h[src] rows (dma_gather, 7 bins = 896 rows
    per call, round-robin over 4 SWDGE queues; >1024-row calls crash the Q7
    ring); per bin, a one-hot/count matrix S (host-built, bf16, exact)
    scatter-sums the gathered rows into B_t[d, dst] in PSUM via TensorE
    matmuls (hi and lo accumulated, exact in fp32 PSUM); then
    aT += W_t^T @ B_t (float32r).  GRU update runs fully on-chip (gates in
    PSUM, sigmoid/tanh on ACT with per-partition bias); graph readout is a
    one-hot matmul + AllReduce.
  - Measured on 8 trn2 cores: ~1.33 ms, rel err 5.7e-4 vs fp32 reference
    (float32r W/GRU matmuls dominate the error; USE_F32R=False gives 6e-6
    at ~2.4 ms).
"""

import numpy as np
import ml_dtypes

import concourse.bacc as bacc
import concourse.mybir as mybir
import concourse.tile as tile
from concourse.masks import make_identity
from concourse.bass_utils import run_bass_kernel_spmd

BF16_NP = ml_dtypes.bfloat16

F32 = mybir.dt.float32
F32R = mybir.dt.float32r
BF16 = mybir.dt.bfloat16
I16 = mybir.dt.int16
AF = mybir.ActivationFunctionType
OP = mybir.AluOpType

HID = 128
USE_F32R = True  # fp32r (tf32-ish) for W/GRU matmuls
MSG_PAIR = False  # single bf16 messages


class Cfg:
    def __init__(self, n_cores, shard, bin_cols, n_etypes, n_steps, n_graphs,
                 n_classes, in_dim):
        assert shard % 128 == 0 and shard % bin_cols == 0
        self.n_cores = n_cores
        self.shard = shard                  # node slots per core
        self.bin = bin_cols                 # node columns per bin
        self.bins = shard // bin_cols       # bins per core
        assert self.bins % 2 == 0
        self.half_bins = self.bins // 2
        self.half_cols = self.half_bins * bin_cols   # node cols per B-half
        assert self.half_cols <= 1024
        self.ntot = n_cores * shard
        self.vpad = ((shard + 511) // 512) * 512     # aT psum width
        self.jt = shard // 128              # 128-wide transpose tiles per core
        self.T = n_etypes
        self.steps = n_steps
        self.G = n_graphs
        self.C = n_classes
        self.in_dim = in_dim
        self.idxc = self.bins * 128 // 16   # idx cols per etype
        self.gbins = 7                      # bins per dma_gather call
        self.scols = self.T * self.bins * self.bin   # S cols total
        self.pw = 2 * 128 if MSG_PAIR else 128      # gather row elems (bf16)


CFG_FULL = dict(n_cores=8, shard=1920, bin_cols=96, n_etypes=13, n_steps=6,
                n_graphs=64, n_classes=10, in_dim=100)


# ---------------------------------------------------------------- host prep

def _pack_nodes(deg, cfg, rng_order=None):
    """Assign each node to a (global bin, slot) s.t. per-(bin,etype) in-edge
    count <= 128 and per-bin node count <= cfg.bin. Returns slot_of[node]."""
    N = deg.shape[0]
    nbins = cfg.n_cores * cfg.bins
    assert N <= nbins * cfg.bin, "not enough node slots"
    used_e = np.zeros((nbins, cfg.T), np.int64)
    used_s = np.zeros(nbins, np.int64)
    order = np.lexsort((-deg.sum(1), -deg.max(1)))
    bin_of = np.empty(N, np.int64)
    for v in order:
        dv = deg[v]
        ok = (used_s < cfg.bin) & ((used_e + dv) <= 128).all(1)
        if not ok.any():
            raise RuntimeError("bin packing failed; reduce bin_cols")
        cand = np.nonzero(ok)[0]
        load = (used_e[cand] + dv).max(1) * 1.0 + used_s[cand] * 0.01
        b = cand[np.argmin(load)]
        used_e[b] += dv
        bin_of[v] = b
        used_s[b] += 1
    # slots within each bin in node order
    slot_of = np.empty(N, np.int64)
    fill = np.zeros(nbins, np.int64)
    for v in range(N):
        b = bin_of[v]
        core, lb = b // cfg.bins, b % cfg.bins
        slot_of[v] = core * cfg.shard + lb * cfg.bin + fill[b]
        fill[b] += 1
    return slot_of


def _hi_lo_pair(x):
    hi = x.astype(BF16_NP)
    lo = (x - hi.astype(np.float32)).astype(BF16_NP)
    return np.concatenate([hi, lo], axis=-1)


def make_plan(feat, src, dst, etypes, graph_ids, W_e, b_e, W_ih, W_hh, b_ih,
              b_hh, W_cls, b_cls, cfg):
    N = feat.shape[0]
    T, S_, B_, BINS = cfg.T, cfg.shard, cfg.bin, cfg.bins
    deg = np.zeros((N, T), np.int64)
    np.add.at(deg, (dst, etypes), 1)
    slot_of = _pack_nodes(deg, cfg)

    # --- edge plan ---
    dslot = slot_of[dst]
    sslot = slot_of[src]
    gbin = dslot // B_                       # global bin (slot space is bin-aligned)
    core = dslot // S_
    lbin = gbin - core * BINS
    tile_id = etypes.astype(np.int64) * BINS + lbin      # per-core tile index
    order = np.lexsort((dslot, tile_id, core))
    c_o, t_o, ds_o, ss_o = core[order], tile_id[order], dslot[order], sslot[order]
    # row index within each (core, tile) group
    key = c_o * (T * BINS) + t_o
    boundaries = np.nonzero(np.diff(key))[0] + 1
    starts = np.concatenate([[0], boundaries])
    group_of = np.searchsorted(starts, np.arange(len(key)), side="right") - 1
    row = np.arange(len(key)) - starts[group_of]
    assert row.max() < 128, "edge cap exceeded (packing bug)"

    NC = cfg.n_cores
    S_host = np.zeros((NC, 128, cfg.scols), np.float32)
    idx_lin = np.zeros((NC, T * BINS, 128), np.int64)
    np.add.at(S_host, (c_o, row, t_o * B_ + (ds_o % B_)), 1.0)
    idx_lin[c_o, t_o, row] = ss_o

    # idx wrap: per etype block of bins*128 positions -> [16, idxc]
    idx_lin = idx_lin.reshape(NC, T, BINS * 128)
    wrapped = idx_lin.reshape(NC, T, cfg.idxc, 16).transpose(0, 3, 1, 2)
    idx_host = np.tile(wrapped.reshape(NC, 16, T * cfg.idxc), (1, 8, 1))
    idx_host = np.ascontiguousarray(idx_host).astype(np.int16)

    # --- degree matrix (for b_e bias), per core [T, vpad] ---
    D_host = np.zeros((NC, T, cfg.vpad), np.float32)
    np.add.at(D_host, (core, etypes.astype(np.int64), dslot % S_), 1.0)

    # --- graph one-hot, per core [128, jt*G] ---
    node_of_slot = np.full(cfg.ntot, -1, np.int64)
    node_of_slot[slot_of] = np.arange(N)
    G_host = np.zeros((NC, 128, cfg.jt * cfg.G), np.float32)
    for c in range(NC):
        sl = node_of_slot[c * S_:(c + 1) * S_]
        valid = np.nonzero(sl >= 0)[0]
        j, p = valid // 128, valid % 128
        g = graph_ids[sl[valid]]
        G_host[c, p, j * cfg.G + g] = 1.0

    # --- h0 ---
    h0 = np.zeros((cfg.ntot, HID), np.float32)
    h0[slot_of, :cfg.in_dim] = feat
    h0_pair = _hi_lo_pair(h0) if MSG_PAIR else h0.astype(BF16_NP)
    h0T = np.zeros((NC, 128, cfg.vpad), np.float32)
    for c in range(NC):
        h0T[c, :, :S_] = h0[c * S_:(c + 1) * S_].T

    # --- weights ---
    W_host = np.ascontiguousarray(W_e.transpose(1, 0, 2).reshape(128, T * HID))
    WihT = np.ascontiguousarray(W_ih.T)             # [128, 384]
    WhhT = np.ascontiguousarray(W_hh.T)
    bias4 = np.stack([
        b_ih[0:HID] + b_hh[0:HID],                  # r
        b_ih[HID:2 * HID] + b_hh[HID:2 * HID],      # z
        b_ih[2 * HID:],                             # n (input side)
        b_hh[2 * HID:],                             # n (hidden side)
    ], axis=1).astype(np.float32)                   # [128, 4]
    WclsT = np.ascontiguousarray(W_cls.T).astype(np.float32)   # [128, C]
    bclsG = np.tile(b_cls[None, :], (cfg.G, 1)).astype(np.float32)

    in_maps = []
    for c in range(NC):
        in_maps.append({
            "h0_pair": h0_pair,
            "h0T": h0T[c],
            "S": S_host[c].astype(BF16_NP),
            "idx": idx_host[c],
            "D": D_host[c],
            "G": G_host[c],
            "W": W_host.astype(np.float32),
            "Wih": WihT.astype(np.float32),
            "Whh": WhhT.astype(np.float32),
            "be": np.ascontiguousarray(b_e).astype(np.float32),
            "bias4": bias4,
            "Wcls": WclsT,
            "bcls": bclsG,
        })
    return in_maps


# ---------------------------------------------------------------- bass build

def _window_pieces(cfg, b):
    """Split bin b's 96-col window at 512-boundaries of its B-half tile.
    Returns (half, [(b_off, width, s_off), ...]) with b_off relative to the
    half tile."""
    half = b // cfg.half_bins
    start = (b - half * cfg.half_bins) * cfg.bin
    end = start + cfg.bin
    pieces = []
    cur = start
    while cur < end:
        nxt = min(end, (cur // 512 + 1) * 512)
        pieces.append((cur, nxt - cur, cur - start))
        cur = nxt
    return half, pieces


def _wmm_pieces(cfg, half):
    """aT col ranges for the W_t matmul of one B half: split the half's node
    cols at 512-boundaries of the aT tile. Returns [(at_off, width, b_off)]."""
    lo = half * cfg.half_cols
    hi = lo + cfg.half_cols
    out = []
    cur = lo
    while cur < hi:
        nxt = min(hi, (cur // 512 + 1) * 512)
        out.append((cur, nxt - cur, cur - lo))
        cur = nxt
    return out


def build_nc(cfg):
    nc = bacc.Bacc("TRN2", target_bir_lowering=False, debug=False,
                   num_devices=cfg.n_cores, num_swdge_queues=4)
    T, BINS, B_, VP, JT = cfg.T, cfg.bins, cfg.bin, cfg.vpad, cfg.jt
    NCH = 512  # gru chunk

    d_pair0 = nc.dram_tensor("h0_pair", [cfg.ntot, cfg.pw], BF16, kind="ExternalInput")
    d_h0T = nc.dram_tensor("h0T", [128, VP], F32, kind="ExternalInput")
    d_S = nc.dram_tensor("S", [128, cfg.scols], BF16, kind="ExternalInput")
    d_idx = nc.dram_tensor("idx", [128, T * cfg.idxc], I16, kind="ExternalInput")
    d_D = nc.dram_tensor("D", [T, VP], F32, kind="ExternalInput")
    d_G = nc.dram_tensor("G", [128, JT * cfg.G], F32, kind="ExternalInput")
    d_W = nc.dram_tensor("W", [128, T * HID], F32, kind="ExternalInput")
    d_Wih = nc.dram_tensor("Wih", [128, 3 * HID], F32, kind="ExternalInput")
    d_Whh = nc.dram_tensor("Whh", [128, 3 * HID], F32, kind="ExternalInput")
    d_be = nc.dram_tensor("be", [T, HID], F32, kind="ExternalInput")
    d_bias4 = nc.dram_tensor("bias4", [128, 4], F32, kind="ExternalInput")
    d_Wcls = nc.dram_tensor("Wcls", [128, cfg.C], F32, kind="ExternalInput")
    d_bcls = nc.dram_tensor("bcls", [cfg.G, cfg.C], F32, kind="ExternalInput")
    d_out = nc.dram_tensor("out", [cfg.G, cfg.C], F32, kind="ExternalOutput")

    # internal dram (collective bounce, double-buffered)
    aspace = "Shared" if cfg.n_cores > 4 else "Local"
    cc_in = [nc.dram_tensor(f"cc_in{i}", [cfg.shard, cfg.pw], BF16)
             for i in range(2)]
    cc_out = [nc.dram_tensor(f"cc_out{i}", [cfg.ntot, cfg.pw], BF16,
                             addr_space=aspace) for i in range(2)]
    hg_in = nc.dram_tensor("hg_in", [cfg.G, HID], F32)
    hg_out = nc.dram_tensor("hg_out", [cfg.G, HID], F32, addr_space=aspace)

    MMDT = F32R if USE_F32R else F32

    with tile.TileContext(nc) as tc:
        def sb(name, shape, dt=F32):
            return nc.alloc_sbuf_tensor(name, list(shape), dt).ap()

        def ps(name, shape, dt=F32):
            return nc.alloc_psum_tensor(name, list(shape), dt).ap()

        S_sb = sb("S_sb", [128, cfg.scols], BF16)
        idx_sb = sb("idx_sb", [128, T * cfg.idxc], I16)
        hT = sb("hT", [128, VP])
        aT_sb = sb("aT_sb", [128, VP], MMDT)
        W_sb = sb("W_sb", [128, T * HID], MMDT)
        Wih_sb = sb("Wih_sb", [128, 3 * HID], MMDT)
        Whh_sb = sb("Whh_sb", [128, 3 * HID], MMDT)
        be_sb = sb("be_sb", [T, HID], MMDT)
        D_sb = sb("D_sb", [T, VP], MMDT)
        bias_sb = sb("bias_sb", [128, 4])
        G_sb = sb("G_sb", [128, JT * cfg.G])
        Wcls_sb = sb("Wcls_sb", [128, cfg.C])
        bcls_sb = sb("bcls_sb", [cfg.G, cfg.C])
        ident = sb("ident", [128, 128])
        h_rows = sb("h_rows", [128, JT * 128])
        hi32 = sb("hi32", [128, JT * 128])
        pair_sb = sb("pair_sb", [128, JT * cfg.pw], BF16)
        hg_sb = sb("hg_sb", [cfg.G, HID])
        hgT_sb = sb("hgT_sb", [128, cfg.G])
        out_sb = sb("out_sb", [cfg.G, cfg.C])
        hTr = sb("hTr", [128, VP], F32R) if USE_F32R else None

        gbuf = [sb(f"gbuf{i}", [128, BINS * cfg.pw], BF16) for i in range(2)]
        Bsb = [sb(f"Bsb{i}", [128, cfg.half_cols], MMDT) for i in range(2)]
        # GRU scratch, 2 sets alternating by chunk parity
        gsc = [{nm: sb(f"gsc{i}_{nm}", [128, NCH])
                for nm in ("r", "z", "hn", "t1", "t2", "n", "d1", "d2")}
               for i in range(2)]

        B_ps = [ps(f"B_ps{i}", [128, 1024]) for i in range(2)]
        aT_ps = ps("aT_ps", [128, VP])

        # ---------------- setup loads ----------------
        if USE_F32R:
            # load fp32 into staging then round via DVE copy into f32r tiles
            stage = sb("stage", [128, T * HID])
            nc.sync.dma_start(stage[:], d_W[:])
            nc.vector.tensor_copy(W_sb[:], stage[:])
            stage2 = sb("stage2", [128, 3 * HID])
            nc.sync.dma_start(stage2[:], d_Wih[:])
            nc.vector.tensor_copy(Wih_sb[:], stage2[:])
            stage3 = sb("stage3", [128, 3 * HID])
            nc.sync.dma_start(stage3[:], d_Whh[:])
            nc.vector.tensor_copy(Whh_sb[:], stage3[:])
            stage4 = sb("stage4", [T, HID])
            nc.sync.dma_start(stage4[:], d_be[:])
            nc.vector.tensor_copy(be_sb[:], stage4[:])
            stage5 = sb("stage5", [T, VP])
            nc.sync.dma_start(stage5[:], d_D[:])
            nc.vector.tensor_copy(D_sb[:], stage5[:])
        else:
            nc.sync.dma_start(W_sb[:], d_W[:])
            nc.sync.dma_start(Wih_sb[:], d_Wih[:])
            nc.sync.dma_start(Whh_sb[:], d_Whh[:])
            nc.sync.dma_start(be_sb[:], d_be[:])
            nc.sync.dma_start(D_sb[:], d_D[:])
        nc.sync.dma_start(idx_sb[:], d_idx[:])
        SC = BINS * B_
        for t in range(T):
            nc.sync.dma_start(S_sb[:, t * SC:(t + 1) * SC],
                              d_S[:, t * SC:(t + 1) * SC])
        nc.sync.dma_start(hT[:], d_h0T[:])
        nc.sync.dma_start(bias_sb[:], d_bias4[:])
        nc.sync.dma_start(G_sb[:], d_G[:])
        nc.sync.dma_start(Wcls_sb[:], d_Wcls[:])
        nc.sync.dma_start(bcls_sb[:], d_bcls[:])
        make_identity(nc, ident[:])

        # ---------------- steps ----------------
        gq = [0]  # rotating SWDGE queue for gathers
        for s in range(cfg.steps):
            pair_src = d_pair0 if s == 0 else cc_out[s % 2]

            # deg * b_e bias: aT = be^T @ D  (start=True covers all of aT)
            for c0 in range(0, VP, 512):
                nc.tensor.matmul(aT_ps[:, c0:c0 + 512], be_sb[:],
                                 D_sb[:, c0:c0 + 512], start=True, stop=False)

            for t in range(T):
                g = gbuf[t % 2]
                g3 = g[:].rearrange("p (b d) -> p b d", d=cfg.pw)
                GB = cfg.gbins
                for b0 in range(0, BINS, GB):
                    nb = min(GB, BINS - b0)
                    nc.gpsimd.dma_gather(
                        g3[:, b0:b0 + nb, :], pair_src[:],
                        idx_sb[:, t * cfg.idxc + b0 * 8:
                               t * cfg.idxc + (b0 + nb) * 8],
                        nb * 128, nb * 128, cfg.pw,
                        queue_num=gq[0] % 4)
                    gq[0] += 1
                for half in range(2):
                    Bp = B_ps[half]
                    # flat entry list: (bank, b_off, w, s_col, lohalf)
                    entries = []
                    for bi in range(cfg.half_bins):
                        b = half * cfg.half_bins + bi
                        _, pieces = _window_pieces(cfg, b)
                        sbase = (t * BINS + b) * B_
                        for (b_off, w, s_off) in pieces:
                            for lo in ((0, 1) if MSG_PAIR else (0,)):
                                entries.append((b_off // 512, b_off, w,
                                                sbase + s_off, lo))
                    first_of = {}
                    last_of = {}
                    for i, e in enumerate(entries):
                        first_of.setdefault(e[0], i)
                        last_of[e[0]] = i
                    for i, (bank, b_off, w, s_col, lo) in enumerate(entries):
                        nc.tensor.matmul(
                            Bp[:, b_off:b_off + w],
                            g3[:, (b_off + half * cfg.half_cols) // B_,
                               lo * HID:(lo + 1) * HID],
                            S_sb[:, s_col:s_col + w],
                            start=(first_of[bank] == i),
                            stop=(last_of[bank] == i))
                    # PSUM -> SBUF (rounds to f32r when enabled)
                    if (t * 2 + half) % 2:
                        nc.scalar.activation(Bsb[half][:, :], Bp[:, :cfg.half_cols],
                                             AF.Identity)
                    else:
                        nc.vector.tensor_copy(Bsb[half][:, :], Bp[:, :cfg.half_cols])
                    # aT += W_t^T @ B_half.  stop=True only on the final
                    # accumulation touching each aT bank (t==T-1; for banks
                    # shared by both halves, only half 1's piece closes it).
                    lhsW = W_sb[:, t * HID:(t + 1) * HID]
                    for (at_off, w, b_off) in _wmm_pieces(cfg, half):
                        bank_end = (at_off + w - 1) // 512
                        shared = (cfg.half_cols % 512) != 0 and \
                            bank_end == cfg.half_cols // 512
                        is_stop = (t == T - 1) and not (half == 0 and shared)
                        nc.tensor.matmul(aT_ps[:, at_off:at_off + w], lhsW,
                                         Bsb[half][:, b_off:b_off + w],
                                         start=False, stop=is_stop)

            gru_rhs_h = hTr if USE_F32R else hT

            # aT psum -> sbuf (chunked; must all finish before odd GRU
            # chunks reuse aT_ps banks for gates)
            for c0 in range(0, VP, NCH):
                nc.scalar.activation(aT_sb[:, c0:c0 + NCH],
                                     aT_ps[:, c0:c0 + NCH], AF.Identity)
                if USE_F32R:
                    nc.vector.tensor_copy(hTr[:, c0:c0 + NCH], hT[:, c0:c0 + NCH])

            # ---------------- GRU ----------------
            for ci, c0 in enumerate(range(0, VP, NCH)):
                if ci % 2 == 0:
                    gA, gB = B_ps[0], B_ps[1]
                else:
                    gA, gB = aT_ps[:, 0:2 * NCH], aT_ps[:, 2 * NCH:4 * NCH]
                r_ps, z_ps = gA[:, 0:NCH], gA[:, NCH:2 * NCH]
                ni_ps, nh_ps = gB[:, 0:NCH], gB[:, NCH:2 * NCH]
                a_c = aT_sb[:, c0:c0 + NCH]
                h_c = gru_rhs_h[:, c0:c0 + NCH]
                nc.tensor.matmul(r_ps, Wih_sb[:, 0:HID], a_c, start=True, stop=False)
                nc.tensor.matmul(r_ps, Whh_sb[:, 0:HID], h_c, start=False, stop=True)
                nc.tensor.matmul(z_ps, Wih_sb[:, HID:2 * HID], a_c, start=True, stop=False)
                nc.tensor.matmul(z_ps, Whh_sb[:, HID:2 * HID], h_c, start=False, stop=True)
                nc.tensor.matmul(ni_ps, Wih_sb[:, 2 * HID:3 * HID], a_c, start=True, stop=True)
                nc.tensor.matmul(nh_ps, Whh_sb[:, 2 * HID:3 * HID], h_c, start=True, stop=True)

                sc = gsc[ci % 2]
                r_sb, z_sb, hn_sb = sc["r"], sc["z"], sc["hn"]
                t1_sb, t2_sb, n_sb = sc["t1"], sc["t2"], sc["n"]
                d1_sb, d2_sb = sc["d1"], sc["d2"]
                nc.scalar.activation(r_sb[:], r_ps, AF.Sigmoid, bias=bias_sb[:, 0:1])
                nc.scalar.activation(z_sb[:], z_ps, AF.Sigmoid, bias=bias_sb[:, 1:2])
                nc.scalar.activation(hn_sb[:], nh_ps, AF.Identity, bias=bias_sb[:, 3:4])
                nc.vector.tensor_tensor(out=t1_sb[:], in0=r_sb[:], in1=hn_sb[:], op=OP.mult)
                nc.vector.tensor_tensor(out=t2_sb[:], in0=t1_sb[:], in1=ni_ps, op=OP.add)
                nc.scalar.activation(n_sb[:], t2_sb[:], AF.Tanh, bias=bias_sb[:, 2:3])
                nc.vector.tensor_tensor(out=d1_sb[:], in0=hT[:, c0:c0 + NCH], in1=n_sb[:], op=OP.subtract)
                nc.vector.tensor_tensor(out=d2_sb[:], in0=d1_sb[:], in1=z_sb[:], op=OP.mult)
                nc.vector.tensor_tensor(out=hT[:, c0:c0 + NCH], in0=d2_sb[:], in1=n_sb[:], op=OP.add)

            # ------- transpose h -> rows; hi/lo split + DMA per 512-range ---
            tp_slots = [B_ps[0][:, 0:128], B_ps[1][:, 0:128],
                        aT_ps[:, 0:128]]
            if VP >= 2048:
                tp_slots.append(aT_ps[:, 1024:1152])
            hr3 = h_rows[:].rearrange("p (j d) -> p j d", d=128)
            pr3 = pair_sb[:].rearrange("p (j d) -> p j d", d=cfg.pw)
            hi3 = hi32[:].rearrange("p (j d) -> p j d", d=128)
            dst = cc_in[(s + 1) % 2]
            dst3 = dst[:].rearrange("(j p) d -> p j d", p=128)
            j_done = 0
            for j in range(JT):
                tp = tp_slots[j % len(tp_slots)]
                nc.tensor.transpose(tp, hT[:, j * 128:(j + 1) * 128], ident[:])
                if j % 2:
                    nc.scalar.activation(h_rows[:, j * 128:(j + 1) * 128], tp, AF.Identity)
                else:
                    nc.vector.tensor_copy(h_rows[:, j * 128:(j + 1) * 128], tp)
                rng_end = (j + 1) * 128
                if s < cfg.steps - 1 and (rng_end % NCH == 0 or j == JT - 1):
                    j0, j1 = j_done, j + 1
                    j_done = j + 1
                    nc.scalar.activation(pr3[:, j0:j1, 0:HID],
                                         hr3[:, j0:j1, :], AF.Identity)
                    if MSG_PAIR:
                        nc.scalar.activation(hi3[:, j0:j1, :],
                                             pr3[:, j0:j1, 0:HID], AF.Identity)
                        nc.vector.tensor_tensor(
                            out=hr3[:, j0:j1, :], in0=hr3[:, j0:j1, :],
                            in1=hi3[:, j0:j1, :], op=OP.subtract)
                        nc.vector.tensor_copy(pr3[:, j0:j1, HID:2 * HID],
                                              hr3[:, j0:j1, :])
                    nc.sync.dma_start(dst3[:, j0:j1, :], pr3[:, j0:j1, :])
            if s < cfg.steps - 1:
                nc.gpsimd.collective_compute(
                    "AllGather", OP.bypass,
                    ins=[dst[:]], outs=[cc_out[(s + 1) % 2][:]],
                    replica_groups=[list(range(cfg.n_cores))])

        # ---------------- readout ----------------
        hg_ps = B_ps[0][0:cfg.G, 0:HID]
        for j in range(JT):
            nc.tensor.matmul(hg_ps, G_sb[:, j * cfg.G:(j + 1) * cfg.G],
                             h_rows[:, j * 128:(j + 1) * 128],
                             start=(j == 0), stop=(j == JT - 1))
        nc.scalar.activation(hg_sb[:], hg_ps, AF.Identity)
        nc.sync.dma_start(hg_in[:], hg_sb[:])
        nc.gpsimd.collective_compute(
            "AllReduce", OP.add, ins=[hg_in[:]], outs=[hg_out[:]],
            replica_groups=[list(range(cfg.n_cores))])
        hg_all = sb("hg_all", [cfg.G, HID])
        nc.sync.dma_start(hg_all[:], hg_out[:])
        tp_ps = B_ps[1][:, 0:cfg.G]
        nc.tensor.transpose(tp_ps, hg_all[:], ident[0:cfg.G, 0:cfg.G])
        nc.vector.tensor_copy(hgT_sb[:], tp_ps)
        lg_ps = B_ps[0][0:cfg.G, 512:512 + cfg.C]
        nc.tensor.matmul(lg_ps, hgT_sb[:], Wcls_sb[:], start=True, stop=True)
        nc.vector.tensor_tensor(out=out_sb[:], in0=lg_ps, in1=bcls_sb[:], op=OP.add)
        nc.sync.dma_start(d_out[:], out_sb[:])

    nc.compile()
    return nc


# ---------------------------------------------------------------- entry

_CACHE = {}
LAST_EXEC_NS = None
LAST_RESULTS = None
PROFILE = False


def _get_nc(cfg_key, cfg):
    if cfg_key not in _CACHE:
        _CACHE[cfg_key] = build_nc(cfg)
    return _CACHE[cfg_key]


def kernel(feat, src, dst, etypes, graph_ids, W_e, b_e, W_ih, W_hh, b_ih,
           b_hh, W_cls, b_cls):
    feat = np.asarray(feat, np.float32)
    args = dict(src=np.asarray(src), dst=np.asarray(dst),
                etypes=np.asarray(etypes), graph_ids=np.asarray(graph_ids),
                W_e=np.asarray(W_e, np.float32), b_e=np.asarray(b_e, np.float32),
                W_ih=np.asarray(W_ih, np.float32), W_hh=np.asarray(W_hh, np.float32),
                b_ih=np.asarray(b_ih, np.float32), b_hh=np.asarray(b_hh, np.float32),
                W_cls=np.asarray(W_cls, np.float32), b_cls=np.asarray(b_cls, np.float32))
    cfg = Cfg(**CFG_FULL)
    in_maps = make_plan(feat=feat, cfg=cfg, **args)
    nc = _get_nc("full", cfg)
    res = run_bass_kernel_spmd(nc, in_maps, list(range(cfg.n_cores)),
                               trace=PROFILE)
    global LAST_EXEC_NS, LAST_RESULTS
    LAST_EXEC_NS = res.exec_time_ns
    LAST_RESULTS = res
    return np.asarray(res.results[0]["out"], np.float32)


# revision 36
# speedup vs baseline: 1.2064x; 1.0034x over previous
"""GGNN (gated graph NN) forward on 8 Trainium2 NeuronCores.

Strategy (node-partitioned, SPMD — one Bass program, per-core data):
  - Nodes are permuted and packed into 8 cores x 20 bins x 96 node-column
    slots such that, for every (bin, etype), the number of in-edges is <= 128.
    This makes the aggregation a fixed static structure: one 128-edge tile per
    (etype, bin).
  - h is kept per-core transposed in SBUF ([128 hid, 2048 node-cols]) for all
    dense matmuls; a row-major bf16 copy lives in DRAM (AllGathered each
    step) and is the source for per-edge dma_gather.  MSG_PAIR=True switches
    to a bf16 hi/lo pair (hi+lo == fp32 h to ~2^-17) for ~4x lower error at
    ~1.2x the time.
  - Per step, per etype t: gather h[src] rows (dma_gather, 7 bins = 896 rows
    per call, round-robin over 4 SWDGE queues; >1024-row calls crash the Q7
    ring); per bin, a one-hot/count matrix S (host-built, bf16, exact)
    scatter-sums the gathered rows into B_t[d, dst] in PSUM via TensorE
    matmuls (hi and lo accumulated, exact in fp32 PSUM); then
    aT += W_t^T @ B_t (float32r).  GRU update runs fully on-chip (gates in
    PSUM, sigmoid/tanh on ACT with per-partition bias); graph readout is a
    one-hot matmul + AllReduce.
  - Measured on 8 trn2 cores: ~1.10 ms, rel err 3.6e-3 vs fp32 reference.
    Accuracy/perf knobs: MSG_PAIR=True -> 1.31 ms @ 5.7e-4; additionally
    USE_F32R=False -> ~2.4 ms @ 6e-6.
"""

import numpy as np
import ml_dtypes

import concourse.bacc as bacc
import concourse.mybir as mybir
import concourse.tile as tile
from concourse.masks import make_identity
from concourse.bass_utils import run_bass_kernel_spmd

BF16_NP = ml_dtypes.bfloat16

F32 = mybir.dt.float32
F32R = mybir.dt.float32r
BF16 = mybir.dt.bfloat16
I16 = mybir.dt.int16
AF = mybir.ActivationFunctionType
OP = mybir.AluOpType

HID = 128
USE_F32R = True  # fp32r (tf32-ish) for W/GRU matmuls
MSG_PAIR = False  # single bf16 messages


class Cfg:
    def __init__(self, n_cores, shard, bin_cols, n_etypes, n_steps, n_graphs,
                 n_classes, in_dim):
        assert shard % 128 == 0 and shard % bin_cols == 0
        self.n_cores = n_cores
        self.shard = shard                  # node slots per core
        self.bin = bin_cols                 # node columns per bin
        self.bins = shard // bin_cols       # bins per core
        assert self.bins % 2 == 0
        self.half_bins = self.bins // 2
        self.half_cols = self.half_bins * bin_cols   # node cols per B-half
        assert self.half_cols <= 1024
        self.ntot = n_cores * shard
        self.vpad = ((shard + 511) // 512) * 512     # aT psum width
        self.jt = shard // 128              # 128-wide transpose tiles per core
        self.T = n_etypes
        self.steps = n_steps
        self.G = n_graphs
        self.C = n_classes
        self.in_dim = in_dim
        self.idxc = self.bins * 128 // 16   # idx cols per etype
        self.gbins = 7                      # bins per dma_gather call
        self.scols = self.T * self.bins * self.bin   # S cols total
        self.pw = 2 * 128 if MSG_PAIR else 128      # gather row elems (bf16)


CFG_FULL = dict(n_cores=8, shard=1920, bin_cols=96, n_etypes=13, n_steps=6,
                n_graphs=64, n_classes=10, in_dim=100)


# ---------------------------------------------------------------- host prep

def _pack_nodes(deg, cfg, rng_order=None):
    """Assign each node to a (global bin, slot) s.t. per-(bin,etype) in-edge
    count <= 128 and per-bin node count <= cfg.bin. Returns slot_of[node]."""
    N = deg.shape[0]
    nbins = cfg.n_cores * cfg.bins
    assert N <= nbins * cfg.bin, "not enough node slots"
    used_e = np.zeros((nbins, cfg.T), np.int64)
    used_s = np.zeros(nbins, np.int64)
    order = np.lexsort((-deg.sum(1), -deg.max(1)))
    bin_of = np.empty(N, np.int64)
    for v in order:
        dv = deg[v]
        ok = (used_s < cfg.bin) & ((used_e + dv) <= 128).all(1)
        if not ok.any():
            raise RuntimeError("bin packing failed; reduce bin_cols")
        cand = np.nonzero(ok)[0]
        load = (used_e[cand] + dv).max(1) * 1.0 + used_s[cand] * 0.01
        b = cand[np.argmin(load)]
        used_e[b] += dv
        bin_of[v] = b
        used_s[b] += 1
    # slots within each bin in node order
    slot_of = np.empty(N, np.int64)
    fill = np.zeros(nbins, np.int64)
    for v in range(N):
        b = bin_of[v]
        core, lb = b // cfg.bins, b % cfg.bins
        slot_of[v] = core * cfg.shard + lb * cfg.bin + fill[b]
        fill[b] += 1
    return slot_of


def _hi_lo_pair(x):
    hi = x.astype(BF16_NP)
    lo = (x - hi.astype(np.float32)).astype(BF16_NP)
    return np.concatenate([hi, lo], axis=-1)


def make_plan(feat, src, dst, etypes, graph_ids, W_e, b_e, W_ih, W_hh, b_ih,
              b_hh, W_cls, b_cls, cfg):
    N = feat.shape[0]
    T, S_, B_, BINS = cfg.T, cfg.shard, cfg.bin, cfg.bins
    deg = np.zeros((N, T), np.int64)
    np.add.at(deg, (dst, etypes), 1)
    slot_of = _pack_nodes(deg, cfg)

    # --- edge plan ---
    dslot = slot_of[dst]
    sslot = slot_of[src]
    gbin = dslot // B_                       # global bin (slot space is bin-aligned)
    core = dslot // S_
    lbin = gbin - core * BINS
    tile_id = etypes.astype(np.int64) * BINS + lbin      # per-core tile index
    order = np.lexsort((dslot, tile_id, core))
    c_o, t_o, ds_o, ss_o = core[order], tile_id[order], dslot[order], sslot[order]
    # row index within each (core, tile) group
    key = c_o * (T * BINS) + t_o
    boundaries = np.nonzero(np.diff(key))[0] + 1
    starts = np.concatenate([[0], boundaries])
    group_of = np.searchsorted(starts, np.arange(len(key)), side="right") - 1
    row = np.arange(len(key)) - starts[group_of]
    assert row.max() < 128, "edge cap exceeded (packing bug)"

    NC = cfg.n_cores
    S_host = np.zeros((NC, 128, cfg.scols), np.float32)
    idx_lin = np.zeros((NC, T * BINS, 128), np.int64)
    np.add.at(S_host, (c_o, row, t_o * B_ + (ds_o % B_)), 1.0)
    idx_lin[c_o, t_o, row] = ss_o

    # idx wrap: per etype block of bins*128 positions -> [16, idxc]
    idx_lin = idx_lin.reshape(NC, T, BINS * 128)
    wrapped = idx_lin.reshape(NC, T, cfg.idxc, 16).transpose(0, 3, 1, 2)
    idx_host = np.tile(wrapped.reshape(NC, 16, T * cfg.idxc), (1, 8, 1))
    idx_host = np.ascontiguousarray(idx_host).astype(np.int16)

    # --- degree matrix (for b_e bias), per core [T, vpad] ---
    D_host = np.zeros((NC, T, cfg.vpad), np.float32)
    np.add.at(D_host, (core, etypes.astype(np.int64), dslot % S_), 1.0)

    # --- graph one-hot, per core [128, jt*G] ---
    node_of_slot = np.full(cfg.ntot, -1, np.int64)
    node_of_slot[slot_of] = np.arange(N)
    G_host = np.zeros((NC, 128, cfg.jt * cfg.G), np.float32)
    for c in range(NC):
        sl = node_of_slot[c * S_:(c + 1) * S_]
        valid = np.nonzero(sl >= 0)[0]
        j, p = valid // 128, valid % 128
        g = graph_ids[sl[valid]]
        G_host[c, p, j * cfg.G + g] = 1.0

    # --- h0 ---
    h0 = np.zeros((cfg.ntot, HID), np.float32)
    h0[slot_of, :cfg.in_dim] = feat
    h0_pair = _hi_lo_pair(h0) if MSG_PAIR else h0.astype(BF16_NP)
    h0T = np.zeros((NC, 128, cfg.vpad), np.float32)
    for c in range(NC):
        h0T[c, :, :S_] = h0[c * S_:(c + 1) * S_].T

    # --- weights ---
    W_host = np.ascontiguousarray(W_e.transpose(1, 0, 2).reshape(128, T * HID))
    WihT = np.ascontiguousarray(W_ih.T)             # [128, 384]
    WhhT = np.ascontiguousarray(W_hh.T)
    bias4 = np.stack([
        b_ih[0:HID] + b_hh[0:HID],                  # r
        b_ih[HID:2 * HID] + b_hh[HID:2 * HID],      # z
        b_ih[2 * HID:],                             # n (input side)
        b_hh[2 * HID:],                             # n (hidden side)
    ], axis=1).astype(np.float32)                   # [128, 4]
    WclsT = np.ascontiguousarray(W_cls.T).astype(np.float32)   # [128, C]
    bclsG = np.tile(b_cls[None, :], (cfg.G, 1)).astype(np.float32)

    in_maps = []
    for c in range(NC):
        in_maps.append({
            "h0_pair": h0_pair,
            "h0T": h0T[c],
            "S": S_host[c].astype(BF16_NP),
            "idx": idx_host[c],
            "D": D_host[c],
            "G": G_host[c],
            "W": W_host.astype(np.float32),
            "Wih": WihT.astype(np.float32),
            "Whh": WhhT.astype(np.float32),
            "be": np.ascontiguousarray(b_e).astype(np.float32),
            "bias4": bias4,
            "Wcls": WclsT,
            "bcls": bclsG,
        })
    return in_maps


# ---------------------------------------------------------------- bass build

def _window_pieces(cfg, b):
    """Split bin b's 96-col window at 512-boundaries of its B-half tile.
    Returns (half, [(b_off, width, s_off), ...]) with b_off relative to the
    half tile."""
    half = b // cfg.half_bins
    start = (b - half * cfg.half_bins) * cfg.bin
    end = start + cfg.bin
    pieces = []
    cur = start
    while cur < end:
        nxt = min(end, (cur // 512 + 1) * 512)
        pieces.append((cur, nxt - cur, cur - start))
        cur = nxt
    return half, pieces


def _wmm_pieces(cfg, half):
    """aT col ranges for the W_t matmul of one B half: split the half's node
    cols at 512-boundaries of the aT tile. Returns [(at_off, width, b_off)]."""
    lo = half * cfg.half_cols
    hi = lo + cfg.half_cols
    out = []
    cur = lo
    while cur < hi:
        nxt = min(hi, (cur // 512 + 1) * 512)
        out.append((cur, nxt - cur, cur - lo))
        cur = nxt
    return out


def build_nc(cfg):
    nc = bacc.Bacc("TRN2", target_bir_lowering=False, debug=False,
                   num_devices=cfg.n_cores, num_swdge_queues=4)
    T, BINS, B_, VP, JT = cfg.T, cfg.bins, cfg.bin, cfg.vpad, cfg.jt
    NCH = 512  # gru chunk

    d_pair0 = nc.dram_tensor("h0_pair", [cfg.ntot, cfg.pw], BF16, kind="ExternalInput")
    d_h0T = nc.dram_tensor("h0T", [128, VP], F32, kind="ExternalInput")
    d_S = nc.dram_tensor("S", [128, cfg.scols], BF16, kind="ExternalInput")
    d_idx = nc.dram_tensor("idx", [128, T * cfg.idxc], I16, kind="ExternalInput")
    d_D = nc.dram_tensor("D", [T, VP], F32, kind="ExternalInput")
    d_G = nc.dram_tensor("G", [128, JT * cfg.G], F32, kind="ExternalInput")
    d_W = nc.dram_tensor("W", [128, T * HID], F32, kind="ExternalInput")
    d_Wih = nc.dram_tensor("Wih", [128, 3 * HID], F32, kind="ExternalInput")
    d_Whh = nc.dram_tensor("Whh", [128, 3 * HID], F32, kind="ExternalInput")
    d_be = nc.dram_tensor("be", [T, HID], F32, kind="ExternalInput")
    d_bias4 = nc.dram_tensor("bias4", [128, 4], F32, kind="ExternalInput")
    d_Wcls = nc.dram_tensor("Wcls", [128, cfg.C], F32, kind="ExternalInput")
    d_bcls = nc.dram_tensor("bcls", [cfg.G, cfg.C], F32, kind="ExternalInput")
    d_out = nc.dram_tensor("out", [cfg.G, cfg.C], F32, kind="ExternalOutput")

    # internal dram (collective bounce, double-buffered)
    aspace = "Shared" if cfg.n_cores > 4 else "Local"
    cc_in = [nc.dram_tensor(f"cc_in{i}", [cfg.shard, cfg.pw], BF16)
             for i in range(2)]
    cc_out = [nc.dram_tensor(f"cc_out{i}", [cfg.ntot, cfg.pw], BF16,
                             addr_space=aspace) for i in range(2)]
    hg_in = nc.dram_tensor("hg_in", [cfg.G, HID], F32)
    hg_out = nc.dram_tensor("hg_out", [cfg.G, HID], F32, addr_space=aspace)

    MMDT = F32R if USE_F32R else F32

    with tile.TileContext(nc) as tc:
        def sb(name, shape, dt=F32):
            return nc.alloc_sbuf_tensor(name, list(shape), dt).ap()

        def ps(name, shape, dt=F32):
            return nc.alloc_psum_tensor(name, list(shape), dt).ap()

        S_sb = sb("S_sb", [128, cfg.scols], BF16)
        idx_sb = sb("idx_sb", [128, T * cfg.idxc], I16)
        hT = sb("hT", [128, VP])
        aT_sb = sb("aT_sb", [128, VP], MMDT)
        W_sb = sb("W_sb", [128, T * HID], MMDT)
        Wih_sb = sb("Wih_sb", [128, 3 * HID], MMDT)
        Whh_sb = sb("Whh_sb", [128, 3 * HID], MMDT)
        be_sb = sb("be_sb", [T, HID], MMDT)
        D_sb = sb("D_sb", [T, VP], MMDT)
        bias_sb = sb("bias_sb", [128, 4])
        G_sb = sb("G_sb", [128, JT * cfg.G])
        Wcls_sb = sb("Wcls_sb", [128, cfg.C])
        bcls_sb = sb("bcls_sb", [cfg.G, cfg.C])
        ident = sb("ident", [128, 128])
        h_rows = sb("h_rows", [128, JT * 128])
        hi32 = sb("hi32", [128, JT * 128])
        pair_sb = sb("pair_sb", [128, JT * cfg.pw], BF16)
        hg_sb = sb("hg_sb", [cfg.G, HID])
        hgT_sb = sb("hgT_sb", [128, cfg.G])
        out_sb = sb("out_sb", [cfg.G, cfg.C])
        hTr = sb("hTr", [128, VP], F32R) if USE_F32R else None

        gbuf = [sb(f"gbuf{i}", [128, BINS * cfg.pw], BF16) for i in range(2)]
        Bsb = [sb(f"Bsb{i}", [128, cfg.half_cols], MMDT) for i in range(2)]
        # GRU scratch, 2 sets alternating by chunk parity
        gsc = [{nm: sb(f"gsc{i}_{nm}", [128, NCH])
                for nm in ("r", "z", "hn", "t1", "t2", "n", "d1", "d2")}
               for i in range(2)]

        B_ps = [ps(f"B_ps{i}", [128, 1024]) for i in range(2)]
        aT_ps = ps("aT_ps", [128, VP])

        # ---------------- setup loads ----------------
        if USE_F32R:
            # load fp32 into staging then round via DVE copy into f32r tiles
            stage = sb("stage", [128, T * HID])
            nc.sync.dma_start(stage[:], d_W[:])
            nc.vector.tensor_copy(W_sb[:], stage[:])
            stage2 = sb("stage2", [128, 3 * HID])
            nc.sync.dma_start(stage2[:], d_Wih[:])
            nc.vector.tensor_copy(Wih_sb[:], stage2[:])
            stage3 = sb("stage3", [128, 3 * HID])
            nc.sync.dma_start(stage3[:], d_Whh[:])
            nc.vector.tensor_copy(Whh_sb[:], stage3[:])
            stage4 = sb("stage4", [T, HID])
            nc.sync.dma_start(stage4[:], d_be[:])
            nc.vector.tensor_copy(be_sb[:], stage4[:])
            stage5 = sb("stage5", [T, VP])
            nc.sync.dma_start(stage5[:], d_D[:])
            nc.vector.tensor_copy(D_sb[:], stage5[:])
        else:
            nc.sync.dma_start(W_sb[:], d_W[:])
            nc.sync.dma_start(Wih_sb[:], d_Wih[:])
            nc.sync.dma_start(Whh_sb[:], d_Whh[:])
            nc.sync.dma_start(be_sb[:], d_be[:])
            nc.sync.dma_start(D_sb[:], d_D[:])
        nc.sync.dma_start(idx_sb[:], d_idx[:])
        SC = BINS * B_
        for t in range(T):
            nc.sync.dma_start(S_sb[:, t * SC:(t + 1) * SC],
                              d_S[:, t * SC:(t + 1) * SC])
        nc.sync.dma_start(hT[:], d_h0T[:])
        nc.sync.dma_start(bias_sb[:], d_bias4[:])
        nc.sync.dma_start(G_sb[:], d_G[:])
        nc.sync.dma_start(Wcls_sb[:], d_Wcls[:])
        nc.sync.dma_start(bcls_sb[:], d_bcls[:])
        make_identity(nc, ident[:])

        # ---------------- steps ----------------
        gq = [0]  # rotating SWDGE queue for gathers
        for s in range(cfg.steps):
            pair_src = d_pair0 if s == 0 else cc_out[s % 2]

            # deg * b_e bias: aT = be^T @ D  (start=True covers all of aT)
            for c0 in range(0, VP, 512):
                nc.tensor.matmul(aT_ps[:, c0:c0 + 512], be_sb[:],
                                 D_sb[:, c0:c0 + 512], start=True, stop=False)

            for t in range(T):
                g = gbuf[t % 2]
                g3 = g[:].rearrange("p (b d) -> p b d", d=cfg.pw)
                GB = cfg.gbins
                for b0 in range(0, BINS, GB):
                    nb = min(GB, BINS - b0)
                    nc.gpsimd.dma_gather(
                        g3[:, b0:b0 + nb, :], pair_src[:],
                        idx_sb[:, t * cfg.idxc + b0 * 8:
                               t * cfg.idxc + (b0 + nb) * 8],
                        nb * 128, nb * 128, cfg.pw,
                        queue_num=gq[0] % 4)
                    gq[0] += 1
                for half in range(2):
                    Bp = B_ps[half]
                    # flat entry list: (bank, b_off, w, s_col, lohalf)
                    entries = []
                    for bi in range(cfg.half_bins):
                        b = half * cfg.half_bins + bi
                        _, pieces = _window_pieces(cfg, b)
                        sbase = (t * BINS + b) * B_
                        for (b_off, w, s_off) in pieces:
                            for lo in ((0, 1) if MSG_PAIR else (0,)):
                                entries.append((b_off // 512, b_off, w,
                                                sbase + s_off, lo))
                    first_of = {}
                    last_of = {}
                    for i, e in enumerate(entries):
                        first_of.setdefault(e[0], i)
                        last_of[e[0]] = i
                    for i, (bank, b_off, w, s_col, lo) in enumerate(entries):
                        nc.tensor.matmul(
                            Bp[:, b_off:b_off + w],
                            g3[:, (b_off + half * cfg.half_cols) // B_,
                               lo * HID:(lo + 1) * HID],
                            S_sb[:, s_col:s_col + w],
                            start=(first_of[bank] == i),
                            stop=(last_of[bank] == i))
                    # PSUM -> SBUF (rounds to f32r when enabled)
                    if (t * 2 + half) % 2:
                        nc.scalar.activation(Bsb[half][:, :], Bp[:, :cfg.half_cols],
                                             AF.Identity)
                    else:
                        nc.vector.tensor_copy(Bsb[half][:, :], Bp[:, :cfg.half_cols])
                    # aT += W_t^T @ B_half.  stop=True only on the final
                    # accumulation touching each aT bank (t==T-1; for banks
                    # shared by both halves, only half 1's piece closes it).
                    lhsW = W_sb[:, t * HID:(t + 1) * HID]
                    for (at_off, w, b_off) in _wmm_pieces(cfg, half):
                        bank_end = (at_off + w - 1) // 512
                        shared = (cfg.half_cols % 512) != 0 and \
                            bank_end == cfg.half_cols // 512
                        is_stop = (t == T - 1) and not (half == 0 and shared)
                        nc.tensor.matmul(aT_ps[:, at_off:at_off + w], lhsW,
                                         Bsb[half][:, b_off:b_off + w],
                                         start=False, stop=is_stop)

            gru_rhs_h = hTr if USE_F32R else hT

            # aT psum -> sbuf (chunked; must all finish before odd GRU
            # chunks reuse aT_ps banks for gates)
            for c0 in range(0, VP, NCH):
                nc.scalar.activation(aT_sb[:, c0:c0 + NCH],
                                     aT_ps[:, c0:c0 + NCH], AF.Identity)
                if USE_F32R:
                    nc.vector.tensor_copy(hTr[:, c0:c0 + NCH], hT[:, c0:c0 + NCH])

            # ---------------- GRU ----------------
            for ci, c0 in enumerate(range(0, VP, NCH)):
                if ci % 2 == 0:
                    gA, gB = B_ps[0], B_ps[1]
                else:
                    gA, gB = aT_ps[:, 0:2 * NCH], aT_ps[:, 2 * NCH:4 * NCH]
                r_ps, z_ps = gA[:, 0:NCH], gA[:, NCH:2 * NCH]
                ni_ps, nh_ps = gB[:, 0:NCH], gB[:, NCH:2 * NCH]
                a_c = aT_sb[:, c0:c0 + NCH]
                h_c = gru_rhs_h[:, c0:c0 + NCH]
                nc.tensor.matmul(r_ps, Wih_sb[:, 0:HID], a_c, start=True, stop=False)
                nc.tensor.matmul(r_ps, Whh_sb[:, 0:HID], h_c, start=False, stop=True)
                nc.tensor.matmul(z_ps, Wih_sb[:, HID:2 * HID], a_c, start=True, stop=False)
                nc.tensor.matmul(z_ps, Whh_sb[:, HID:2 * HID], h_c, start=False, stop=True)
                nc.tensor.matmul(ni_ps, Wih_sb[:, 2 * HID:3 * HID], a_c, start=True, stop=True)
                nc.tensor.matmul(nh_ps, Whh_sb[:, 2 * HID:3 * HID], h_c, start=True, stop=True)

                sc = gsc[ci % 2]
                r_sb, z_sb, hn_sb = sc["r"], sc["z"], sc["hn"]
                t1_sb, t2_sb, n_sb = sc["t1"], sc["t2"], sc["n"]
                d1_sb, d2_sb = sc["d1"], sc["d2"]
                nc.scalar.activation(r_sb[:], r_ps, AF.Sigmoid, bias=bias_sb[:, 0:1])
                nc.scalar.activation(z_sb[:], z_ps, AF.Sigmoid, bias=bias_sb[:, 1:2])
                nc.scalar.activation(hn_sb[:], nh_ps, AF.Identity, bias=bias_sb[:, 3:4])
                nc.vector.tensor_tensor(out=t1_sb[:], in0=r_sb[:], in1=hn_sb[:], op=OP.mult)
                nc.vector.tensor_tensor(out=t2_sb[:], in0=t1_sb[:], in1=ni_ps, op=OP.add)
                nc.scalar.activation(n_sb[:], t2_sb[:], AF.Tanh, bias=bias_sb[:, 2:3])
                nc.vector.tensor_tensor(out=d1_sb[:], in0=hT[:, c0:c0 + NCH], in1=n_sb[:], op=OP.subtract)
                nc.vector.tensor_tensor(out=d2_sb[:], in0=d1_sb[:], in1=z_sb[:], op=OP.mult)
                nc.vector.tensor_tensor(out=hT[:, c0:c0 + NCH], in0=d2_sb[:], in1=n_sb[:], op=OP.add)

            # ------- transpose h -> rows; hi/lo split + DMA per 512-range ---
            tp_slots = [B_ps[0][:, 0:128], B_ps[1][:, 0:128],
                        aT_ps[:, 0:128]]
            if VP >= 2048:
                tp_slots.append(aT_ps[:, 1024:1152])
            hr3 = h_rows[:].rearrange("p (j d) -> p j d", d=128)
            pr3 = pair_sb[:].rearrange("p (j d) -> p j d", d=cfg.pw)
            hi3 = hi32[:].rearrange("p (j d) -> p j d", d=128)
            dst = cc_in[(s + 1) % 2]
            dst3 = dst[:].rearrange("(j p) d -> p j d", p=128)
            j_done = 0
            for j in range(JT):
                tp = tp_slots[j % len(tp_slots)]
                nc.tensor.transpose(tp, hT[:, j * 128:(j + 1) * 128], ident[:])
                if j % 2:
                    nc.scalar.activation(h_rows[:, j * 128:(j + 1) * 128], tp, AF.Identity)
                else:
                    nc.vector.tensor_copy(h_rows[:, j * 128:(j + 1) * 128], tp)
                rng_end = (j + 1) * 128
                if s < cfg.steps - 1 and (rng_end % NCH == 0 or j == JT - 1):
                    j0, j1 = j_done, j + 1
                    j_done = j + 1
                    nc.scalar.activation(pr3[:, j0:j1, 0:HID],
                                         hr3[:, j0:j1, :], AF.Identity)
                    if MSG_PAIR:
                        nc.scalar.activation(hi3[:, j0:j1, :],
                                             pr3[:, j0:j1, 0:HID], AF.Identity)
                        nc.vector.tensor_tensor(
                            out=hr3[:, j0:j1, :], in0=hr3[:, j0:j1, :],
                            in1=hi3[:, j0:j1, :], op=OP.subtract)
                        nc.vector.tensor_copy(pr3[:, j0:j1, HID:2 * HID],
                                              hr3[:, j0:j1, :])
                    nc.sync.dma_start(dst3[:, j0:j1, :], pr3[:, j0:j1, :])
            if s < cfg.steps - 1:
                nc.gpsimd.collective_compute(
                    "AllGather", OP.bypass,
                    ins=[dst[:]], outs=[cc_out[(s + 1) % 2][:]],
                    replica_groups=[list(range(cfg.n_cores))])

        # ---------------- readout ----------------
        hg_ps = B_ps[0][0:cfg.G, 0:HID]
        for j in range(JT):
            nc.tensor.matmul(hg_ps, G_sb[:, j * cfg.G:(j + 1) * cfg.G],
                             h_rows[:, j * 128:(j + 1) * 128],
                             start=(j == 0), stop=(j == JT - 1))
        nc.scalar.activation(hg_sb[:], hg_ps, AF.Identity)
        nc.sync.dma_start(hg_in[:], hg_sb[:])
        nc.gpsimd.collective_compute(
            "AllReduce", OP.add, ins=[hg_in[:]], outs=[hg_out[:]],
            replica_groups=[list(range(cfg.n_cores))])
        hg_all = sb("hg_all", [cfg.G, HID])
        nc.sync.dma_start(hg_all[:], hg_out[:])
        tp_ps = B_ps[1][:, 0:cfg.G]
        nc.tensor.transpose(tp_ps, hg_all[:], ident[0:cfg.G, 0:cfg.G])
        nc.vector.tensor_copy(hgT_sb[:], tp_ps)
        lg_ps = B_ps[0][0:cfg.G, 512:512 + cfg.C]
        nc.tensor.matmul(lg_ps, hgT_sb[:], Wcls_sb[:], start=True, stop=True)
        nc.vector.tensor_tensor(out=out_sb[:], in0=lg_ps, in1=bcls_sb[:], op=OP.add)
        nc.sync.dma_start(d_out[:], out_sb[:])

    nc.compile()
    return nc


# ---------------------------------------------------------------- entry

_CACHE = {}
LAST_EXEC_NS = None
LAST_RESULTS = None
PROFILE = False


def _get_nc(cfg_key, cfg):
    if cfg_key not in _CACHE:
        _CACHE[cfg_key] = build_nc(cfg)
    return _CACHE[cfg_key]


def kernel(feat, src, dst, etypes, graph_ids, W_e, b_e, W_ih, W_hh, b_ih,
           b_hh, W_cls, b_cls):
    feat = np.asarray(feat, np.float32)
    args = dict(src=np.asarray(src), dst=np.asarray(dst),
                etypes=np.asarray(etypes), graph_ids=np.asarray(graph_ids),
                W_e=np.asarray(W_e, np.float32), b_e=np.asarray(b_e, np.float32),
                W_ih=np.asarray(W_ih, np.float32), W_hh=np.asarray(W_hh, np.float32),
                b_ih=np.asarray(b_ih, np.float32), b_hh=np.asarray(b_hh, np.float32),
                W_cls=np.asarray(W_cls, np.float32), b_cls=np.asarray(b_cls, np.float32))
    cfg = Cfg(**CFG_FULL)
    in_maps = make_plan(feat=feat, cfg=cfg, **args)
    nc = _get_nc("full", cfg)
    res = run_bass_kernel_spmd(nc, in_maps, list(range(cfg.n_cores)),
                               trace=PROFILE)
    global LAST_EXEC_NS, LAST_RESULTS
    LAST_EXEC_NS = res.exec_time_ns
    LAST_RESULTS = res
    return np.asarray(res.results[0]["out"], np.float32)


# revision 37
# speedup vs baseline: 1.2281x; 1.0180x over previous
"""GGNN (gated graph NN) forward on 8 Trainium2 NeuronCores.

Strategy (node-partitioned, SPMD — one Bass program, per-core data):
  - Nodes are permuted and packed into 8 cores x 20 bins x 96 node-column
    slots such that, for every (bin, etype), the number of in-edges is <= 128.
    This makes the aggregation a fixed static structure: one 128-edge tile per
    (etype, bin).
  - h is kept per-core transposed in SBUF ([128 hid, 2048 node-cols]) for all
    dense matmuls; a row-major bf16 copy lives in DRAM (AllGathered each
    step) and is the source for per-edge dma_gather.  MSG_PAIR=True switches
    to a bf16 hi/lo pair (hi+lo == fp32 h to ~2^-17) for ~4x lower error at
    ~1.2x the time.
  - Per step, per etype t: gather h[src] rows (dma_gather, 7 bins = 896 rows
    per call, round-robin over 4 SWDGE queues; >1024-row calls crash the Q7
    ring); per bin, a one-hot/count matrix S (host-built, bf16, exact)
    scatter-sums the gathered rows into B_t[d, dst] in PSUM via TensorE
    matmuls (hi and lo accumulated, exact in fp32 PSUM); then
    aT += W_t^T @ B_t (float32r).  GRU update runs fully on-chip (gates in
    PSUM, sigmoid/tanh on ACT with per-partition bias); graph readout is a
    one-hot matmul + AllReduce.
  - Measured on 8 trn2 cores: ~1.10 ms, rel err 3.6e-3 vs fp32 reference.
    Accuracy/perf knobs: MSG_PAIR=True -> 1.31 ms @ 5.7e-4; additionally
    USE_F32R=False -> ~2.4 ms @ 6e-6.
"""

import numpy as np
import ml_dtypes

import concourse.bacc as bacc
import concourse.mybir as mybir
import concourse.tile as tile
from concourse.masks import make_identity
from concourse.bass_utils import run_bass_kernel_spmd

BF16_NP = ml_dtypes.bfloat16

F32 = mybir.dt.float32
F32R = mybir.dt.float32r
BF16 = mybir.dt.bfloat16
I16 = mybir.dt.int16
AF = mybir.ActivationFunctionType
OP = mybir.AluOpType

HID = 128
USE_F32R = True  # fp32r (tf32-ish) for W/GRU matmuls
MSG_PAIR = False  # single bf16 messages


class Cfg:
    def __init__(self, n_cores, shard, bin_cols, n_etypes, n_steps, n_graphs,
                 n_classes, in_dim):
        assert shard % 128 == 0 and shard % bin_cols == 0
        self.n_cores = n_cores
        self.shard = shard                  # node slots per core
        self.bin = bin_cols                 # node columns per bin
        self.bins = shard // bin_cols       # bins per core
        assert self.bins % 2 == 0
        self.half_bins = self.bins // 2
        self.half_cols = self.half_bins * bin_cols   # node cols per B-half
        assert self.half_cols <= 1024
        self.ntot = n_cores * shard
        self.vpad = ((shard + 511) // 512) * 512     # aT psum width
        self.jt = shard // 128              # 128-wide transpose tiles per core
        self.T = n_etypes
        self.steps = n_steps
        self.G = n_graphs
        self.C = n_classes
        self.in_dim = in_dim
        self.idxc = self.bins * 128 // 16   # idx cols per etype
        self.gbins = 7                      # bins per dma_gather call
        self.scols = self.T * self.bins * self.bin   # S cols total
        self.pw = 2 * 128 if MSG_PAIR else 128      # gather row elems (bf16)


CFG_FULL = dict(n_cores=8, shard=1920, bin_cols=96, n_etypes=13, n_steps=6,
                n_graphs=64, n_classes=10, in_dim=100)


# ---------------------------------------------------------------- host prep

def _pack_nodes(deg, cfg, rng_order=None):
    """Assign each node to a (global bin, slot) s.t. per-(bin,etype) in-edge
    count <= 128 and per-bin node count <= cfg.bin. Returns slot_of[node]."""
    N = deg.shape[0]
    nbins = cfg.n_cores * cfg.bins
    assert N <= nbins * cfg.bin, "not enough node slots"
    used_e = np.zeros((nbins, cfg.T), np.int64)
    used_s = np.zeros(nbins, np.int64)
    order = np.lexsort((-deg.sum(1), -deg.max(1)))
    bin_of = np.empty(N, np.int64)
    for v in order:
        dv = deg[v]
        ok = (used_s < cfg.bin) & ((used_e + dv) <= 128).all(1)
        if not ok.any():
            raise RuntimeError("bin packing failed; reduce bin_cols")
        cand = np.nonzero(ok)[0]
        load = (used_e[cand] + dv).max(1) * 1.0 + used_s[cand] * 0.01
        b = cand[np.argmin(load)]
        used_e[b] += dv
        bin_of[v] = b
        used_s[b] += 1
    # slots within each bin in node order
    slot_of = np.empty(N, np.int64)
    fill = np.zeros(nbins, np.int64)
    for v in range(N):
        b = bin_of[v]
        core, lb = b // cfg.bins, b % cfg.bins
        slot_of[v] = core * cfg.shard + lb * cfg.bin + fill[b]
        fill[b] += 1
    return slot_of


def _hi_lo_pair(x):
    hi = x.astype(BF16_NP)
    lo = (x - hi.astype(np.float32)).astype(BF16_NP)
    return np.concatenate([hi, lo], axis=-1)


def make_plan(feat, src, dst, etypes, graph_ids, W_e, b_e, W_ih, W_hh, b_ih,
              b_hh, W_cls, b_cls, cfg):
    N = feat.shape[0]
    T, S_, B_, BINS = cfg.T, cfg.shard, cfg.bin, cfg.bins
    deg = np.zeros((N, T), np.int64)
    np.add.at(deg, (dst, etypes), 1)
    slot_of = _pack_nodes(deg, cfg)

    # --- edge plan ---
    dslot = slot_of[dst]
    sslot = slot_of[src]
    gbin = dslot // B_                       # global bin (slot space is bin-aligned)
    core = dslot // S_
    lbin = gbin - core * BINS
    tile_id = etypes.astype(np.int64) * BINS + lbin      # per-core tile index
    order = np.lexsort((dslot, tile_id, core))
    c_o, t_o, ds_o, ss_o = core[order], tile_id[order], dslot[order], sslot[order]
    # row index within each (core, tile) group
    key = c_o * (T * BINS) + t_o
    boundaries = np.nonzero(np.diff(key))[0] + 1
    starts = np.concatenate([[0], boundaries])
    group_of = np.searchsorted(starts, np.arange(len(key)), side="right") - 1
    row = np.arange(len(key)) - starts[group_of]
    assert row.max() < 128, "edge cap exceeded (packing bug)"

    NC = cfg.n_cores
    S_host = np.zeros((NC, 128, cfg.scols), np.float32)
    idx_lin = np.zeros((NC, T * BINS, 128), np.int64)
    np.add.at(S_host, (c_o, row, t_o * B_ + (ds_o % B_)), 1.0)
    idx_lin[c_o, t_o, row] = ss_o

    # idx wrap: per etype block of bins*128 positions -> [16, idxc]
    idx_lin = idx_lin.reshape(NC, T, BINS * 128)
    wrapped = idx_lin.reshape(NC, T, cfg.idxc, 16).transpose(0, 3, 1, 2)
    idx_host = np.tile(wrapped.reshape(NC, 16, T * cfg.idxc), (1, 8, 1))
    idx_host = np.ascontiguousarray(idx_host).astype(np.int16)

    # --- degree matrix (for b_e bias), per core [T, vpad] ---
    D_host = np.zeros((NC, T, cfg.vpad), np.float32)
    np.add.at(D_host, (core, etypes.astype(np.int64), dslot % S_), 1.0)

    # --- graph one-hot, per core [128, jt*G] ---
    node_of_slot = np.full(cfg.ntot, -1, np.int64)
    node_of_slot[slot_of] = np.arange(N)
    G_host = np.zeros((NC, 128, cfg.jt * cfg.G), np.float32)
    for c in range(NC):
        sl = node_of_slot[c * S_:(c + 1) * S_]
        valid = np.nonzero(sl >= 0)[0]
        j, p = valid // 128, valid % 128
        g = graph_ids[sl[valid]]
        G_host[c, p, j * cfg.G + g] = 1.0

    # --- h0 ---
    h0 = np.zeros((cfg.ntot, HID), np.float32)
    h0[slot_of, :cfg.in_dim] = feat
    h0_pair = _hi_lo_pair(h0) if MSG_PAIR else h0.astype(BF16_NP)
    h0T = np.zeros((NC, 128, cfg.vpad), np.float32)
    for c in range(NC):
        h0T[c, :, :S_] = h0[c * S_:(c + 1) * S_].T

    # --- weights ---
    W_host = np.ascontiguousarray(W_e.transpose(1, 0, 2).reshape(128, T * HID))
    WihT = np.ascontiguousarray(W_ih.T)             # [128, 384]
    WhhT = np.ascontiguousarray(W_hh.T)
    bias4 = np.stack([
        b_ih[0:HID] + b_hh[0:HID],                  # r
        b_ih[HID:2 * HID] + b_hh[HID:2 * HID],      # z
        b_ih[2 * HID:],                             # n (input side)
        b_hh[2 * HID:],                             # n (hidden side)
    ], axis=1).astype(np.float32)                   # [128, 4]
    WclsT = np.ascontiguousarray(W_cls.T).astype(np.float32)   # [128, C]
    bclsG = np.tile(b_cls[None, :], (cfg.G, 1)).astype(np.float32)

    in_maps = []
    for c in range(NC):
        in_maps.append({
            "h0_pair": h0_pair,
            "h0T": h0T[c],
            "S": S_host[c].astype(BF16_NP),
            "idx": idx_host[c],
            "D": D_host[c],
            "G": G_host[c],
            "W": W_host.astype(np.float32),
            "Wih": WihT.astype(np.float32),
            "Whh": WhhT.astype(np.float32),
            "be": np.ascontiguousarray(b_e).astype(np.float32),
            "bias4": bias4,
            "Wcls": WclsT,
            "bcls": bclsG,
        })
    return in_maps


# ---------------------------------------------------------------- bass build

def _window_pieces(cfg, b):
    """Split bin b's 96-col window at 512-boundaries of its B-half tile.
    Returns (half, [(b_off, width, s_off), ...]) with b_off relative to the
    half tile."""
    half = b // cfg.half_bins
    start = (b - half * cfg.half_bins) * cfg.bin
    end = start + cfg.bin
    pieces = []
    cur = start
    while cur < end:
        nxt = min(end, (cur // 512 + 1) * 512)
        pieces.append((cur, nxt - cur, cur - start))
        cur = nxt
    return half, pieces


def _wmm_pieces(cfg, half):
    """aT col ranges for the W_t matmul of one B half: split the half's node
    cols at 512-boundaries of the aT tile. Returns [(at_off, width, b_off)]."""
    lo = half * cfg.half_cols
    hi = lo + cfg.half_cols
    out = []
    cur = lo
    while cur < hi:
        nxt = min(hi, (cur // 512 + 1) * 512)
        out.append((cur, nxt - cur, cur - lo))
        cur = nxt
    return out


def build_nc(cfg):
    nc = bacc.Bacc("TRN2", target_bir_lowering=False, debug=False,
                   num_devices=cfg.n_cores, num_swdge_queues=4)
    T, BINS, B_, VP, JT = cfg.T, cfg.bins, cfg.bin, cfg.vpad, cfg.jt
    NCH = 512  # gru chunk

    d_pair0 = nc.dram_tensor("h0_pair", [cfg.ntot, cfg.pw], BF16, kind="ExternalInput")
    d_h0T = nc.dram_tensor("h0T", [128, VP], F32, kind="ExternalInput")
    d_S = nc.dram_tensor("S", [128, cfg.scols], BF16, kind="ExternalInput")
    d_idx = nc.dram_tensor("idx", [128, T * cfg.idxc], I16, kind="ExternalInput")
    d_D = nc.dram_tensor("D", [T, VP], F32, kind="ExternalInput")
    d_G = nc.dram_tensor("G", [128, JT * cfg.G], F32, kind="ExternalInput")
    d_W = nc.dram_tensor("W", [128, T * HID], F32, kind="ExternalInput")
    d_Wih = nc.dram_tensor("Wih", [128, 3 * HID], F32, kind="ExternalInput")
    d_Whh = nc.dram_tensor("Whh", [128, 3 * HID], F32, kind="ExternalInput")
    d_be = nc.dram_tensor("be", [T, HID], F32, kind="ExternalInput")
    d_bias4 = nc.dram_tensor("bias4", [128, 4], F32, kind="ExternalInput")
    d_Wcls = nc.dram_tensor("Wcls", [128, cfg.C], F32, kind="ExternalInput")
    d_bcls = nc.dram_tensor("bcls", [cfg.G, cfg.C], F32, kind="ExternalInput")
    d_out = nc.dram_tensor("out", [cfg.G, cfg.C], F32, kind="ExternalOutput")

    # internal dram (collective bounce, double-buffered)
    aspace = "Shared" if cfg.n_cores > 4 else "Local"
    cc_in = [nc.dram_tensor(f"cc_in{i}", [cfg.shard, cfg.pw], BF16)
             for i in range(2)]
    cc_out = [nc.dram_tensor(f"cc_out{i}", [cfg.ntot, cfg.pw], BF16,
                             addr_space=aspace) for i in range(2)]
    hg_in = nc.dram_tensor("hg_in", [cfg.G, HID], F32)
    hg_out = nc.dram_tensor("hg_out", [cfg.G, HID], F32, addr_space=aspace)

    MMDT = F32R if USE_F32R else F32

    with tile.TileContext(nc) as tc:
        def sb(name, shape, dt=F32):
            return nc.alloc_sbuf_tensor(name, list(shape), dt).ap()

        def ps(name, shape, dt=F32):
            return nc.alloc_psum_tensor(name, list(shape), dt).ap()

        S_sb = sb("S_sb", [128, cfg.scols], BF16)
        idx_sb = sb("idx_sb", [128, T * cfg.idxc], I16)
        hT = sb("hT", [128, VP])
        aT_sb = sb("aT_sb", [128, VP], MMDT)
        W_sb = sb("W_sb", [128, T * HID], MMDT)
        Wih_sb = sb("Wih_sb", [128, 3 * HID], MMDT)
        Whh_sb = sb("Whh_sb", [128, 3 * HID], MMDT)
        be_sb = sb("be_sb", [T, HID], MMDT)
        D_sb = sb("D_sb", [T, VP], MMDT)
        bias_sb = sb("bias_sb", [128, 4])
        G_sb = sb("G_sb", [128, JT * cfg.G])
        Wcls_sb = sb("Wcls_sb", [128, cfg.C])
        bcls_sb = sb("bcls_sb", [cfg.G, cfg.C])
        ident = sb("ident", [128, 128])
        h_rows = sb("h_rows", [128, JT * 128])
        hi32 = sb("hi32", [128, JT * 128])
        pair_sb = sb("pair_sb", [128, JT * cfg.pw], BF16)
        hg_sb = sb("hg_sb", [cfg.G, HID])
        hgT_sb = sb("hgT_sb", [128, cfg.G])
        out_sb = sb("out_sb", [cfg.G, cfg.C])
        hTr = sb("hTr", [128, VP], F32R) if USE_F32R else None

        gbuf = [sb(f"gbuf{i}", [128, BINS * cfg.pw], BF16) for i in range(4)]
        Bsb = [sb(f"Bsb{i}", [128, cfg.half_cols], MMDT) for i in range(2)]
        # GRU scratch, 2 sets alternating by chunk parity
        gsc = [{nm: sb(f"gsc{i}_{nm}", [128, NCH])
                for nm in ("r", "z", "hn", "t1", "t2", "n", "d1", "d2")}
               for i in range(2)]

        B_ps = [ps(f"B_ps{i}", [128, 1024]) for i in range(2)]
        aT_ps = ps("aT_ps", [128, VP])

        # ---------------- setup loads ----------------
        if USE_F32R:
            # load fp32 into staging then round via DVE copy into f32r tiles
            stage = sb("stage", [128, T * HID])
            nc.sync.dma_start(stage[:], d_W[:])
            nc.vector.tensor_copy(W_sb[:], stage[:])
            stage2 = sb("stage2", [128, 3 * HID])
            nc.sync.dma_start(stage2[:], d_Wih[:])
            nc.vector.tensor_copy(Wih_sb[:], stage2[:])
            stage3 = sb("stage3", [128, 3 * HID])
            nc.sync.dma_start(stage3[:], d_Whh[:])
            nc.vector.tensor_copy(Whh_sb[:], stage3[:])
            stage4 = sb("stage4", [T, HID])
            nc.sync.dma_start(stage4[:], d_be[:])
            nc.vector.tensor_copy(be_sb[:], stage4[:])
            stage5 = sb("stage5", [T, VP])
            nc.sync.dma_start(stage5[:], d_D[:])
            nc.vector.tensor_copy(D_sb[:], stage5[:])
        else:
            nc.sync.dma_start(W_sb[:], d_W[:])
            nc.sync.dma_start(Wih_sb[:], d_Wih[:])
            nc.sync.dma_start(Whh_sb[:], d_Whh[:])
            nc.sync.dma_start(be_sb[:], d_be[:])
            nc.sync.dma_start(D_sb[:], d_D[:])
        nc.sync.dma_start(idx_sb[:], d_idx[:])
        SC = BINS * B_
        for t in range(T):
            nc.sync.dma_start(S_sb[:, t * SC:(t + 1) * SC],
                              d_S[:, t * SC:(t + 1) * SC])
        nc.sync.dma_start(hT[:], d_h0T[:])
        nc.sync.dma_start(bias_sb[:], d_bias4[:])
        nc.sync.dma_start(G_sb[:], d_G[:])
        nc.sync.dma_start(Wcls_sb[:], d_Wcls[:])
        nc.sync.dma_start(bcls_sb[:], d_bcls[:])
        make_identity(nc, ident[:])

        # ---------------- steps ----------------
        gq = [0]  # rotating SWDGE queue for gathers
        for s in range(cfg.steps):
            pair_src = d_pair0 if s == 0 else cc_out[s % 2]

            # deg * b_e bias: aT = be^T @ D  (start=True covers all of aT)
            for c0 in range(0, VP, 512):
                nc.tensor.matmul(aT_ps[:, c0:c0 + 512], be_sb[:],
                                 D_sb[:, c0:c0 + 512], start=True, stop=False)

            for t in range(T):
                g = gbuf[t % 4]
                g3 = g[:].rearrange("p (b d) -> p b d", d=cfg.pw)
                GB = cfg.gbins
                for b0 in range(0, BINS, GB):
                    nb = min(GB, BINS - b0)
                    nc.gpsimd.dma_gather(
                        g3[:, b0:b0 + nb, :], pair_src[:],
                        idx_sb[:, t * cfg.idxc + b0 * 8:
                               t * cfg.idxc + (b0 + nb) * 8],
                        nb * 128, nb * 128, cfg.pw,
                        queue_num=gq[0] % 4)
                    gq[0] += 1
                for half in range(2):
                    Bp = B_ps[half]
                    # flat entry list: (bank, b_off, w, s_col, lohalf)
                    entries = []
                    for bi in range(cfg.half_bins):
                        b = half * cfg.half_bins + bi
                        _, pieces = _window_pieces(cfg, b)
                        sbase = (t * BINS + b) * B_
                        for (b_off, w, s_off) in pieces:
                            for lo in ((0, 1) if MSG_PAIR else (0,)):
                                entries.append((b_off // 512, b_off, w,
                                                sbase + s_off, lo))
                    first_of = {}
                    last_of = {}
                    for i, e in enumerate(entries):
                        first_of.setdefault(e[0], i)
                        last_of[e[0]] = i
                    for i, (bank, b_off, w, s_col, lo) in enumerate(entries):
                        nc.tensor.matmul(
                            Bp[:, b_off:b_off + w],
                            g3[:, (b_off + half * cfg.half_cols) // B_,
                               lo * HID:(lo + 1) * HID],
                            S_sb[:, s_col:s_col + w],
                            start=(first_of[bank] == i),
                            stop=(last_of[bank] == i))
                    # PSUM -> SBUF (rounds to f32r when enabled)
                    if (t * 2 + half) % 2:
                        nc.scalar.activation(Bsb[half][:, :], Bp[:, :cfg.half_cols],
                                             AF.Identity)
                    else:
                        nc.vector.tensor_copy(Bsb[half][:, :], Bp[:, :cfg.half_cols])
                    # aT += W_t^T @ B_half.  stop=True only on the final
                    # accumulation touching each aT bank (t==T-1; for banks
                    # shared by both halves, only half 1's piece closes it).
                    lhsW = W_sb[:, t * HID:(t + 1) * HID]
                    for (at_off, w, b_off) in _wmm_pieces(cfg, half):
                        bank_end = (at_off + w - 1) // 512
                        shared = (cfg.half_cols % 512) != 0 and \
                            bank_end == cfg.half_cols // 512
                        is_stop = (t == T - 1) and not (half == 0 and shared)
                        nc.tensor.matmul(aT_ps[:, at_off:at_off + w], lhsW,
                                         Bsb[half][:, b_off:b_off + w],
                                         start=False, stop=is_stop)

            gru_rhs_h = hTr if USE_F32R else hT

            # aT psum -> sbuf (chunked; must all finish before odd GRU
            # chunks reuse aT_ps banks for gates)
            for c0 in range(0, VP, NCH):
                nc.scalar.activation(aT_sb[:, c0:c0 + NCH],
                                     aT_ps[:, c0:c0 + NCH], AF.Identity)
                if USE_F32R:
                    nc.vector.tensor_copy(hTr[:, c0:c0 + NCH], hT[:, c0:c0 + NCH])

            # ---------------- GRU ----------------
            for ci, c0 in enumerate(range(0, VP, NCH)):
                if ci % 2 == 0:
                    gA, gB = B_ps[0], B_ps[1]
                else:
                    gA, gB = aT_ps[:, 0:2 * NCH], aT_ps[:, 2 * NCH:4 * NCH]
                r_ps, z_ps = gA[:, 0:NCH], gA[:, NCH:2 * NCH]
                ni_ps, nh_ps = gB[:, 0:NCH], gB[:, NCH:2 * NCH]
                a_c = aT_sb[:, c0:c0 + NCH]
                h_c = gru_rhs_h[:, c0:c0 + NCH]
                nc.tensor.matmul(r_ps, Wih_sb[:, 0:HID], a_c, start=True, stop=False)
                nc.tensor.matmul(r_ps, Whh_sb[:, 0:HID], h_c, start=False, stop=True)
                nc.tensor.matmul(z_ps, Wih_sb[:, HID:2 * HID], a_c, start=True, stop=False)
                nc.tensor.matmul(z_ps, Whh_sb[:, HID:2 * HID], h_c, start=False, stop=True)
                nc.tensor.matmul(ni_ps, Wih_sb[:, 2 * HID:3 * HID], a_c, start=True, stop=True)
                nc.tensor.matmul(nh_ps, Whh_sb[:, 2 * HID:3 * HID], h_c, start=True, stop=True)

                sc = gsc[ci % 2]
                r_sb, z_sb, hn_sb = sc["r"], sc["z"], sc["hn"]
                t1_sb, t2_sb, n_sb = sc["t1"], sc["t2"], sc["n"]
                d1_sb, d2_sb = sc["d1"], sc["d2"]
                nc.scalar.activation(r_sb[:], r_ps, AF.Sigmoid, bias=bias_sb[:, 0:1])
                nc.scalar.activation(z_sb[:], z_ps, AF.Sigmoid, bias=bias_sb[:, 1:2])
                nc.scalar.activation(hn_sb[:], nh_ps, AF.Identity, bias=bias_sb[:, 3:4])
                nc.vector.tensor_tensor(out=t1_sb[:], in0=r_sb[:], in1=hn_sb[:], op=OP.mult)
                nc.vector.tensor_tensor(out=t2_sb[:], in0=t1_sb[:], in1=ni_ps, op=OP.add)
                nc.scalar.activation(n_sb[:], t2_sb[:], AF.Tanh, bias=bias_sb[:, 2:3])
                nc.vector.tensor_tensor(out=d1_sb[:], in0=hT[:, c0:c0 + NCH], in1=n_sb[:], op=OP.subtract)
                nc.vector.tensor_tensor(out=d2_sb[:], in0=d1_sb[:], in1=z_sb[:], op=OP.mult)
                nc.vector.tensor_tensor(out=hT[:, c0:c0 + NCH], in0=d2_sb[:], in1=n_sb[:], op=OP.add)

            # ------- transpose h -> rows; hi/lo split + DMA per 512-range ---
            tp_slots = [B_ps[0][:, 0:128], B_ps[1][:, 0:128],
                        aT_ps[:, 0:128]]
            if VP >= 2048:
                tp_slots.append(aT_ps[:, 1024:1152])
            hr3 = h_rows[:].rearrange("p (j d) -> p j d", d=128)
            pr3 = pair_sb[:].rearrange("p (j d) -> p j d", d=cfg.pw)
            hi3 = hi32[:].rearrange("p (j d) -> p j d", d=128)
            dst = cc_in[(s + 1) % 2]
            dst3 = dst[:].rearrange("(j p) d -> p j d", p=128)
            j_done = 0
            for j in range(JT):
                tp = tp_slots[j % len(tp_slots)]
                nc.tensor.transpose(tp, hT[:, j * 128:(j + 1) * 128], ident[:])
                if j % 2:
                    nc.scalar.activation(h_rows[:, j * 128:(j + 1) * 128], tp, AF.Identity)
                else:
                    nc.vector.tensor_copy(h_rows[:, j * 128:(j + 1) * 128], tp)
                rng_end = (j + 1) * 128
                if s < cfg.steps - 1 and (rng_end % NCH == 0 or j == JT - 1):
                    j0, j1 = j_done, j + 1
                    j_done = j + 1
                    nc.scalar.activation(pr3[:, j0:j1, 0:HID],
                                         hr3[:, j0:j1, :], AF.Identity)
                    if MSG_PAIR:
                        nc.scalar.activation(hi3[:, j0:j1, :],
                                             pr3[:, j0:j1, 0:HID], AF.Identity)
                        nc.vector.tensor_tensor(
                            out=hr3[:, j0:j1, :], in0=hr3[:, j0:j1, :],
                            in1=hi3[:, j0:j1, :], op=OP.subtract)
                        nc.vector.tensor_copy(pr3[:, j0:j1, HID:2 * HID],
                                              hr3[:, j0:j1, :])
                    nc.sync.dma_start(dst3[:, j0:j1, :], pr3[:, j0:j1, :])
            if s < cfg.steps - 1:
                nc.gpsimd.collective_compute(
                    "AllGather", OP.bypass,
                    ins=[dst[:]], outs=[cc_out[(s + 1) % 2][:]],
                    replica_groups=[list(range(cfg.n_cores))])

        # ---------------- readout ----------------
        hg_ps = B_ps[0][0:cfg.G, 0:HID]
        for j in range(JT):
            nc.tensor.matmul(hg_ps, G_sb[:, j * cfg.G:(j + 1) * cfg.G],
                             h_rows[:, j * 128:(j + 1) * 128],
                             start=(j == 0), stop=(j == JT - 1))
        nc.scalar.activation(hg_sb[:], hg_ps, AF.Identity)
        nc.sync.dma_start(hg_in[:], hg_sb[:])
        nc.gpsimd.collective_compute(
            "AllReduce", OP.add, ins=[hg_in[:]], outs=[hg_out[:]],
            replica_groups=[list(range(cfg.n_cores))])
        hg_all = sb("hg_all", [cfg.G, HID])
        nc.sync.dma_start(hg_all[:], hg_out[:])
        tp_ps = B_ps[1][:, 0:cfg.G]
        nc.tensor.transpose(tp_ps, hg_all[:], ident[0:cfg.G, 0:cfg.G])
        nc.vector.tensor_copy(hgT_sb[:], tp_ps)
        lg_ps = B_ps[0][0:cfg.G, 512:512 + cfg.C]
        nc.tensor.matmul(lg_ps, hgT_sb[:], Wcls_sb[:], start=True, stop=True)
        nc.vector.tensor_tensor(out=out_sb[:], in0=lg_ps, in1=bcls_sb[:], op=OP.add)
        nc.sync.dma_start(d_out[:], out_sb[:])

    nc.compile()
    return nc


# ---------------------------------------------------------------- entry

_CACHE = {}
LAST_EXEC_NS = None
LAST_RESULTS = None
PROFILE = False


def _get_nc(cfg_key, cfg):
    if cfg_key not in _CACHE:
        _CACHE[cfg_key] = build_nc(cfg)
    return _CACHE[cfg_key]


def kernel(feat, src, dst, etypes, graph_ids, W_e, b_e, W_ih, W_hh, b_ih,
           b_hh, W_cls, b_cls):
    feat = np.asarray(feat, np.float32)
    args = dict(src=np.asarray(src), dst=np.asarray(dst),
                etypes=np.asarray(etypes), graph_ids=np.asarray(graph_ids),
                W_e=np.asarray(W_e, np.float32), b_e=np.asarray(b_e, np.float32),
                W_ih=np.asarray(W_ih, np.float32), W_hh=np.asarray(W_hh, np.float32),
                b_ih=np.asarray(b_ih, np.float32), b_hh=np.asarray(b_hh, np.float32),
                W_cls=np.asarray(W_cls, np.float32), b_cls=np.asarray(b_cls, np.float32))
    cfg = Cfg(**CFG_FULL)
    in_maps = make_plan(feat=feat, cfg=cfg, **args)
    nc = _get_nc("full", cfg)
    res = run_bass_kernel_spmd(nc, in_maps, list(range(cfg.n_cores)),
                               trace=PROFILE)
    global LAST_EXEC_NS, LAST_RESULTS
    LAST_EXEC_NS = res.exec_time_ns
    LAST_RESULTS = res
    return np.asarray(res.results[0]["out"], np.float32)
